# revision 6
# baseline (speedup 1.0000x reference)
# Distributed Trainium2 (8 NeuronCore) Bass kernel for nn_ArtOutBlock.
#
# Sharding: data-parallel over batch (16 batches -> 2 per core) for the heavy
# pointnet conv + max-pool phase; exact BatchNorm batch statistics via three
# small AllGathers of per-core partial bn_stats (+ pooled values), aggregated
# on-device with bn_aggr.
import os
import sys
import types

sys.path.insert(0, "/opt/trn_rl_repo")

import numpy as np
import ml_dtypes

# --- NTFF profile hook (antenv.axon_hooks is stubbed out in this image) ----
import antenv  # noqa: E402

if "antenv.axon_hooks" not in sys.modules:
    _m = types.ModuleType("antenv.axon_hooks")
    _m._hook = None
    _m.set_axon_ntff_profile_hook = lambda h: setattr(_m, "_hook", h)
    _m.get_axon_ntff_profile_hook = lambda: _m._hook
    sys.modules["antenv.axon_hooks"] = _m
    antenv.axon_hooks = _m

try:
    from trn_agent_boot.trn_boot import _ntff_profile_via_ctypes

    sys.modules["antenv.axon_hooks"].set_axon_ntff_profile_hook(
        _ntff_profile_via_ctypes("/opt/axon/libaxon_pjrt.so")
    )
except Exception:
    pass

import concourse.bass as bass  # noqa: E402,F401
import concourse.bacc as bacc  # noqa: E402
import concourse.mybir as mybir  # noqa: E402
import concourse.tile as tile  # noqa: E402
from concourse import bass_utils  # noqa: E402
from concourse.masks import make_identity  # noqa: E402

bass_utils.upload_artifacts = lambda tmpdir: f"file://{tmpdir}"

F32 = mybir.dt.float32
BF16 = mybir.dt.bfloat16
I32 = mybir.dt.int32
AX = mybir.AxisListType
ALU = mybir.AluOpType
ACTF = mybir.ActivationFunctionType

N_CORES = 8
NB, NP, NA, NROT = 16, 1024, 12, 60
C0, C1 = 64, 128
CIN, CFEAT, COUT = 256, 512, 256
BPC = NB // N_CORES  # batches per core = 2
NT = BPC * NROT      # 120
EPS = 1e-5

# bounce1 per-rank layout (f32 words)
B1_P0 = 0            # pn0 pooled, per b: 64*12      -> 2*768
B1_P1 = 1536         # pn1 pooled, per b: 128*12     -> 2*1536
B1_S0 = 4608         # pn0 bn_stats, per b: 64*6     -> 2*384
B1_S1 = 5376         # pn1 bn_stats, per b: 128*6    -> 2*768
B1_N = 6912
B2_N = 4 * 128 * 6
B3_N = 6 * 128 * 6

WEIGHT_SPECS = [
    ("w_pn0f", (128, 128), BF16), ("w_pn0x", (6, 128), BF16),
    ("w_pn1f", (128, 128), BF16), ("w_pn1x", (3, 128), BF16),
    ("g_pn0", (64, 1), F32), ("bb_pn0", (64, 1), F32),
    ("g_pn1", (128, 1), F32), ("bb_pn1", (128, 1), F32),
    ("w_lin0", (64, 256), F32), ("w_lin1", (128, 256), F32),
    ("g_lin", (128, 2), F32), ("bb_lin", (128, 2), F32),
    ("ti", (1, 720), I32),
    ("w_feat", (128, 2 * NA * CFEAT), BF16),
    ("g_feat", (128, 4), F32), ("bb_feat", (128, 4), F32),
    ("w_reg1", (128, 512), F32),
    ("g_reg", (128, 1), F32), ("bb_reg", (128, 1), F32),
    ("w_att1", (128, 512), F32),
    ("g_att", (128, 1), F32), ("bb_att", (128, 1), F32),
    ("w_reg2", (128, 7), F32), ("b_reg2", (7, 1), F32),
    ("w_att2", (128, 1), F32), ("b_att2", (1, 1), F32),
    ("w_out1", (128, 2048), F32),
    ("g_out", (128, 4), F32), ("bb_out", (128, 4), F32),
    ("w_out2", (128, 1024), F32), ("b_out2", (128, 2), F32),
    ("anch", (NT, 9), F32),
]


def _build(debug_outs=False):
    nc = bacc.Bacc("TRN2", target_bir_lowering=False, debug=False,
                   num_devices=N_CORES)
    params = {}

    def P(name, shape, dt):
        params[name] = nc.declare_dram_parameter(name, list(shape), dt,
                                                 isOutput=False)

    P("f0", (BPC, 128, 6144), BF16)   # (b, (h,c), (p',a))
    P("f1", (BPC, 128, NP * NA), BF16)
    P("x0", (BPC, 6, NP // 2), BF16)  # (b, (h,c), p')
    P("x1", (BPC, 3, NP), BF16)
    for name, shape, dt in WEIGHT_SPECS:
        P(name, shape, dt)

    outs = {
        "x_out": nc.declare_dram_parameter("x_out", [BPC, COUT], F32, True),
        "x_attn": nc.declare_dram_parameter("x_attn", [BPC, NROT], F32, True),
        "pred_R": nc.declare_dram_parameter("pred_R", [BPC, NROT, 3, 3], F32,
                                            True),
        "res_T": nc.declare_dram_parameter("res_T", [BPC, NROT, 3], F32, True),
    }
    if debug_outs:
        outs["dbg_pool"] = nc.declare_dram_parameter(
            "dbg_pool", [BPC, 192, NA], F32, True)
        outs["dbg_xf"] = nc.declare_dram_parameter(
            "dbg_xf", [128, NT], F32, True)
        outs["dbg_lin"] = nc.declare_dram_parameter(
            "dbg_lin", [BPC, NA, 256], F32, True)

    with tile.TileContext(nc) as tc:
        _graph(nc, tc, params, outs)
    nc.finalize()
    return nc


def _graph(nc, tc, prm, outs):
    from contextlib import ExitStack

    RG = [list(range(N_CORES))]
    ctx = ExitStack()
    with ctx:
        consts = ctx.enter_context(tc.tile_pool(name="consts", bufs=1))
        wpool = ctx.enter_context(tc.tile_pool(name="wpool", bufs=1))
        small = ctx.enter_context(tc.tile_pool(name="small", bufs=1))
        dram = ctx.enter_context(tc.tile_pool(name="dram", bufs=1,
                                              space="DRAM"))

        ident = consts.tile([128, 128], F32, tag="ident")
        make_identity(nc, ident)

        # register float constants used as activation biases
        for cv in (EPS, float(np.pi / 2.0), float(-np.pi / 10.0)):
            ct = consts.tile([128, 1], F32, tag=f"cst{cv}")
            nc.gpsimd.memset(ct, cv)
            nc.const_aps.aps[(F32, cv)] = ct

        W = {}
        for name, shape, dt in WEIGHT_SPECS:
            tl = wpool.tile(list(shape), dt, tag=name)
            nc.sync.dma_start(tl, prm[name].ap())
            W[name] = tl
        x0t, x1t = [], []
        for b in range(BPC):
            xt = wpool.tile([6, 512], BF16, tag=f"x0_{b}")
            nc.sync.dma_start(xt, prm["x0"].ap()[b])
            x0t.append(xt)
            yt = wpool.tile([3, 1024], BF16, tag=f"x1_{b}")
            nc.sync.dma_start(yt, prm["x1"].ap()[b])
            x1t.append(yt)

        # ================= phase 1: convs + max pool =================
        bigin = ctx.enter_context(tc.tile_pool(name="bigin", bufs=3))
        pools = {}

        def conv_level(kind, b):
            if kind == "pn0":
                src = prm["f0"].ap()[b]      # (128, 6144)
                ncols, nslots = 6144, 4
                wf, wx, xt = W["w_pn0f"], W["w_pn0x"], x0t[b]
            else:
                src = prm["f1"].ap()[b]      # (128, 12288)
                ncols, nslots = NP * NA, 8
                wf, wx, xt = W["w_pn1f"], W["w_pn1x"], x1t[b]

            inp = bigin.tile([128, ncols], BF16, tag="conv_in")
            nc.sync.dma_start(inp, src)
            slots = small.tile([128, nslots * NA], F32,
                               tag=f"slots_{kind}_{b}")
            with tc.tile_pool(name=f"cps_{kind}_{b}", bufs=2,
                              space="PSUM") as cpp:
                for s in range(nslots):  # each slot: 128 points
                    ps = cpp.tile([128, 2048], F32, tag="cpsum")
                    for m in range(4):   # 4 matmuls x 32 points
                        pbase = s * 128 + m * 32
                        nc.tensor.matmul(ps[:, m * 512:m * 512 + 384],
                                         wf, inp[:, pbase * NA:(pbase + 32) * NA],
                                         start=True, stop=False)
                        rx = xt[:, pbase:pbase + 32].unsqueeze(2) \
                            .broadcast_to([xt.shape[0], 32, NA])
                        nc.tensor.matmul(ps[:, m * 512:m * 512 + 384],
                                         wx, rx, start=False, stop=True)
                    red = ps.rearrange("q (m c) -> q m c", m=4)[:, :, 0:384] \
                        .rearrange("q m (p a) -> q m p a", a=NA) \
                        .transpose([0, 3, 1, 2])  # (128, 12, 4, 32)
                    nc.vector.tensor_reduce(slots[:, s * NA:(s + 1) * NA],
                                            red, axis=AX.XY, op=ALU.max)
            pooled = small.tile([128, NA], F32, tag=f"pool_{kind}_{b}")
            nc.vector.tensor_reduce(
                pooled,
                slots.rearrange("q (s a) -> q s a", a=NA).transpose([0, 2, 1]),
                axis=AX.X, op=ALU.max)
            return pooled

        for b in range(BPC):
            pools[("pn1", b)] = conv_level("pn1", b)
        for b in range(BPC):
            pools[("pn0", b)] = conv_level("pn0", b)

        # pn0: combine point-halves (partitions (h,o)) via transpose
        p0 = {}
        with tc.tile_pool(name="tps", bufs=2, space="PSUM") as tps:
            for b in range(BPC):
                pt = tps.tile([NA, 128], F32, tag="tp")
                nc.tensor.transpose(pt, pools[("pn0", b)], ident)
                pts = small.tile([NA, 128], F32, tag=f"pts{b}")
                nc.scalar.copy(pts, pt)
                hm = small.tile([NA, 64], F32, tag=f"hmax{b}")
                nc.vector.tensor_tensor(hm, pts[:, 0:64], pts[:, 64:128],
                                        op=ALU.max)
                bt = tps.tile([64, NA], F32, tag="tpb")
                nc.tensor.transpose(bt, hm, ident[0:NA, 0:NA])
                p0b = small.tile([64, NA], F32, tag=f"p0_{b}")
                nc.scalar.copy(p0b, bt)
                p0[b] = p0b

        stats1 = {}
        for b in range(BPC):
            s0 = small.tile([64, 6], F32, tag=f"s0_{b}")
            nc.vector.bn_stats(s0, p0[b])
            stats1[("pn0", b)] = s0
            s1 = small.tile([128, 6], F32, tag=f"s1_{b}")
            nc.vector.bn_stats(s1, pools[("pn1", b)])
            stats1[("pn1", b)] = s1

        # ---- bounce1 + AllGather 1 ----
        b1in = dram.tile([B1_N], F32, tag="b1in")
        b1out = dram.tile([N_CORES, B1_N], F32, tag="b1out")
        for b in range(BPC):
            nc.sync.dma_start(
                b1in[B1_P0 + b * 768:B1_P0 + (b + 1) * 768]
                .rearrange("(c a) -> c a", a=NA), p0[b])
            nc.sync.dma_start(
                b1in[B1_P1 + b * 1536:B1_P1 + (b + 1) * 1536]
                .rearrange("(c a) -> c a", a=NA), pools[("pn1", b)])
            nc.sync.dma_start(
                b1in[B1_S0 + b * 384:B1_S0 + (b + 1) * 384]
                .rearrange("(c k) -> c k", k=6), stats1[("pn0", b)])
            nc.sync.dma_start(
                b1in[B1_S1 + b * 768:B1_S1 + (b + 1) * 768]
                .rearrange("(c k) -> c k", k=6), stats1[("pn1", b)])
        nc.gpsimd.collective_compute(
            "AllGather", ALU.bypass, replica_groups=RG,
            ins=[b1in.opt()], outs=[b1out.opt()])
        g1 = b1out  # (8, B1_N)

        def agg_from(srcs, parts, tag):
            # srcs: list of 3-dim (parts, 8, 6) dram views
            st = small.tile([parts, len(srcs), 8, 6], F32, tag=f"aggin_{tag}",
                            name=f"aggin_{tag}")
            for i, sv in enumerate(srcs):
                nc.sync.dma_start(st[:, i], sv)
            ag = small.tile([parts, 2], F32, tag=f"agg_{tag}",
                            name=f"agg_{tag}")
            nc.vector.bn_aggr(ag, st)
            return ag

        def stat_view(off, nchan, b):
            return g1[:, off + b * nchan * 6:off + (b + 1) * nchan * 6] \
                .rearrange("r (c k) -> r c k", k=6).transpose([1, 0, 2])

        agg_pn0 = agg_from([stat_view(B1_S0, 64, b) for b in range(2)],
                           64, "pn0")
        agg_pn1 = agg_from([stat_view(B1_S1, 128, b) for b in range(2)],
                           128, "pn1")

        def mk_scale_shift(agg, g_t, bb_t, parts, tag):
            sd = small.tile([parts, 1], F32, tag=f"sd_{tag}")
            nc.scalar.activation(sd, agg[:, 1:2], ACTF.Sqrt, bias=EPS)
            rs = small.tile([parts, 1], F32, tag=f"rs_{tag}")
            nc.vector.reciprocal(rs, sd)
            s = small.tile([parts, 1], F32, tag=f"s_{tag}")
            nc.vector.tensor_tensor(s, rs, g_t, op=ALU.mult)
            ms = small.tile([parts, 1], F32, tag=f"ms_{tag}")
            nc.vector.tensor_tensor(ms, agg[:, 0:1], s, op=ALU.mult)
            sh = small.tile([parts, 1], F32, tag=f"sh_{tag}")
            nc.vector.tensor_tensor(sh, bb_t, ms, op=ALU.subtract)
            return s, sh

        s_pn0, t_pn0 = mk_scale_shift(agg_pn0, W["g_pn0"], W["bb_pn0"],
                                      64, "pn0")
        s_pn1, t_pn1 = mk_scale_shift(agg_pn1, W["g_pn1"], W["bb_pn1"],
                                      128, "pn1")

        # ---- all-batch pooled -> lin stats ----
        tileA = small.tile([64, 2, 8, NA], F32, tag="tileA")
        tileB = small.tile([128, 2, 8, NA], F32, tag="tileB")
        for b in range(2):
            nc.sync.dma_start(
                tileA[:, b],
                g1[:, B1_P0 + b * 768:B1_P0 + (b + 1) * 768]
                .rearrange("r (c a) -> r c a", a=NA).transpose([1, 0, 2]))
            nc.sync.dma_start(
                tileB[:, b],
                g1[:, B1_P1 + b * 1536:B1_P1 + (b + 1) * 1536]
                .rearrange("r (c a) -> r c a", a=NA).transpose([1, 0, 2]))
        nc.scalar.activation(tileA, tileA, ACTF.Relu, scale=s_pn0, bias=t_pn0)
        nc.scalar.activation(tileB, tileB, ACTF.Relu, scale=s_pn1, bias=t_pn1)

        ps2 = ctx.enter_context(tc.tile_pool(name="ps2", bufs=4, space="PSUM"))
        psf = ctx.enter_context(tc.tile_pool(name="psf", bufs=4, space="PSUM"))

        lin_s, lin_t = [], []
        for oc in range(2):
            pl = ps2.tile([128, 192], F32, tag="mm")
            nc.tensor.matmul(pl, W["w_lin0"][:, oc * 128:(oc + 1) * 128],
                             tileA.rearrange("c b r a -> c (b r a)"),
                             start=True, stop=False)
            nc.tensor.matmul(pl, W["w_lin1"][:, oc * 128:(oc + 1) * 128],
                             tileB.rearrange("c b r a -> c (b r a)"),
                             start=False, stop=True)
            st = small.tile([128, 6], F32, tag=f"linst{oc}")
            nc.vector.bn_stats(st, pl)
            ag = small.tile([128, 2], F32, tag=f"linag{oc}")
            nc.vector.bn_aggr(ag, st)
            s2, t2 = mk_scale_shift(ag, W["g_lin"][:, oc:oc + 1],
                                    W["bb_lin"][:, oc:oc + 1],
                                    128, f"lin{oc}")
            lin_s.append(s2)
            lin_t.append(t2)

        # ---- own-batch pooled BN -> lin -> transpose ----
        p0bn, p1bn = {}, {}
        for b in range(BPC):
            a0 = small.tile([64, NA], F32, tag=f"p0bn{b}")
            nc.scalar.activation(a0, p0[b], ACTF.Relu, scale=s_pn0, bias=t_pn0)
            p0bn[b] = a0
            a1 = small.tile([128, NA], F32, tag=f"p1bn{b}")
            nc.scalar.activation(a1, pools[("pn1", b)], ACTF.Relu,
                                 scale=s_pn1, bias=t_pn1)
            p1bn[b] = a1

        linT = {b: small.tile([NA, 256], BF16, tag=f"linT{b}",
                               name=f"linT{b}") for b in range(BPC)}
        lin_bn_dbg = {}
        for b in range(BPC):
            for oc in range(2):
                pl = ps2.tile([128, NA], F32, tag="mm")
                nc.tensor.matmul(pl, W["w_lin0"][:, oc * 128:(oc + 1) * 128],
                                 p0bn[b], start=True, stop=False)
                nc.tensor.matmul(pl, W["w_lin1"][:, oc * 128:(oc + 1) * 128],
                                 p1bn[b], start=False, stop=True)
                lb = small.tile([128, NA], F32, tag=f"linbn{b}{oc}")
                nc.scalar.activation(lb, pl, ACTF.Identity,
                                     scale=lin_s[oc], bias=lin_t[oc])
                lin_bn_dbg[(b, oc)] = lb
                pt = ps2.tile([NA, 128], F32, tag="mm")
                nc.tensor.transpose(pt, lb, ident)
                nc.scalar.copy(linT[b][:, oc * 128:(oc + 1) * 128], pt)

        # ---- one-hot from trace_idx ----
        ti_f1 = small.tile([1, 720], F32, tag="ti_f1")
        nc.vector.tensor_copy(ti_f1, W["ti"])
        ti_f = small.tile([NA, 720], F32, tag="ti_f")
        nc.gpsimd.partition_broadcast(ti_f, ti_f1)
        io_i = small.tile([NA, 720], I32, tag="io_i")
        nc.gpsimd.iota(io_i, pattern=[[0, 720]], base=0, channel_multiplier=1)
        io_f = small.tile([NA, 720], F32, tag="io_f")
        nc.vector.tensor_copy(io_f, io_i)
        oh = small.tile([NA, 720], BF16, tag="oh")
        nc.vector.tensor_tensor(oh, ti_f, io_f, op=ALU.is_equal)

        # ---- gather matmuls -> X2 ----
        X2 = [small.tile([128, BPC * 720], BF16, tag=f"X2_{cc}",
                         name=f"X2_{cc}") for cc in range(2)]
        for b in range(BPC):
            for cc in range(2):
                for j in range(2):
                    pg = ps2.tile([128, 360], F32, tag="mm")
                    nc.tensor.matmul(pg, linT[b][:, cc * 128:(cc + 1) * 128],
                                     oh[:, j * 360:(j + 1) * 360],
                                     start=True, stop=True)
                    nc.scalar.copy(
                        X2[cc][:, b * 720 + j * 360:b * 720 + (j + 1) * 360],
                        pg)

        # ---- feat matmuls ----
        wf_v = W["w_feat"].rearrange("q (cc a o) -> q cc a o", cc=2, a=NA)
        feat_ps = []
        for oc in range(4):
            pf = psf.tile([128, NT], F32, tag="featmm")
            first = True
            for cc in range(2):
                for a in range(NA):
                    rhs = X2[cc].rearrange("q (b a r) -> q b a r",
                                           b=BPC, a=NA)[:, :, a, :]
                    nc.tensor.matmul(pf,
                                     wf_v[:, cc, a, oc * 128:(oc + 1) * 128],
                                     rhs, start=first,
                                     stop=(cc == 1 and a == NA - 1))
                    first = False
            feat_ps.append(pf)

        # ---- feat stats -> AllGather 2 ----
        b2in = dram.tile([B2_N], F32, tag="b2in")
        b2out = dram.tile([N_CORES, B2_N], F32, tag="b2out")
        for oc in range(4):
            fs = small.tile([128, 6], F32, tag=f"fstat{oc}")
            nc.vector.bn_stats(fs, feat_ps[oc])
            nc.sync.dma_start(
                b2in[oc * 768:(oc + 1) * 768].rearrange("(c k) -> c k", k=6),
                fs)
        nc.gpsimd.collective_compute(
            "AllGather", ALU.bypass, replica_groups=RG,
            ins=[b2in.opt()], outs=[b2out.opt()])

        xf, xm = [], []
        for oc in range(4):
            ag = agg_from(
                [b2out[:, oc * 768:(oc + 1) * 768]
                 .rearrange("r (c k) -> r c k", k=6).transpose([1, 0, 2])],
                128, f"feat{oc}")
            s3, t3 = mk_scale_shift(ag, W["g_feat"][:, oc:oc + 1],
                                    W["bb_feat"][:, oc:oc + 1],
                                    128, f"feat{oc}")
            x = small.tile([128, NT], F32, tag=f"xf{oc}")
            nc.scalar.activation(x, feat_ps[oc], ACTF.Relu, scale=s3, bias=t3)
            xf.append(x)
            m = small.tile([128, BPC], F32, tag=f"xm{oc}")
            nc.vector.tensor_reduce(m, x.rearrange("q (b r) -> q b r", b=BPC),
                                    axis=AX.X, op=ALU.max)
            xm.append(m)

        # ---- reg1 / att1 / out1 ----
        wr1 = W["w_reg1"].rearrange("q (k o) -> q k o", k=4)
        wa1 = W["w_att1"].rearrange("q (k o) -> q k o", k=4)
        wo1 = W["w_out1"].rearrange("q (k o) -> q k o", k=4)
        p_reg = ps2.tile([128, NT], F32, tag="mm")
        p_att = ps2.tile([128, NT], F32, tag="mm")
        for k in range(4):
            nc.tensor.matmul(p_reg, wr1[:, k, :], xf[k],
                             start=(k == 0), stop=(k == 3))
        for k in range(4):
            nc.tensor.matmul(p_att, wa1[:, k, :], xf[k],
                             start=(k == 0), stop=(k == 3))
        p_out1 = []
        for oc in range(4):
            po = psf.tile([128, BPC], F32, tag="featmm")
            for k in range(4):
                nc.tensor.matmul(po, wo1[:, k, oc * 128:(oc + 1) * 128],
                                 xm[k], start=(k == 0), stop=(k == 3))
            p_out1.append(po)

        # ---- stats -> AllGather 3 ----
        b3in = dram.tile([B3_N], F32, tag="b3in")
        b3out = dram.tile([N_CORES, B3_N], F32, tag="b3out")

        def put_stats(ps_t, off, tag):
            st = small.tile([128, 6], F32, tag=f"st3_{tag}")
            nc.vector.bn_stats(st, ps_t)
            nc.sync.dma_start(
                b3in[off:off + 768].rearrange("(c k) -> c k", k=6), st)

        put_stats(p_reg, 0, "reg")
        put_stats(p_att, 768, "att")
        for oc in range(4):
            put_stats(p_out1[oc], 1536 + oc * 768, f"o1{oc}")
        nc.gpsimd.collective_compute(
            "AllGather", ALU.bypass, replica_groups=RG,
            ins=[b3in.opt()], outs=[b3out.opt()])

        def agg3(off, g_t, bb_t, tag):
            ag = agg_from(
                [b3out[:, off:off + 768]
                 .rearrange("r (c k) -> r c k", k=6).transpose([1, 0, 2])],
                128, f"a3_{tag}")
            return mk_scale_shift(ag, g_t, bb_t, 128, f"h_{tag}")

        s4, t4 = agg3(0, W["g_reg"], W["bb_reg"], "reg")
        s5, t5 = agg3(768, W["g_att"], W["bb_att"], "att")
        h_reg = small.tile([128, NT], F32, tag="h_reg")
        nc.scalar.activation(h_reg, p_reg, ACTF.Relu, scale=s4, bias=t4)
        a_att = small.tile([128, NT], F32, tag="a_att")
        nc.scalar.activation(a_att, p_att, ACTF.Relu, scale=s5, bias=t5)
        o_o = []
        for oc in range(4):
            s6, t6 = agg3(1536 + oc * 768, W["g_out"][:, oc:oc + 1],
                          W["bb_out"][:, oc:oc + 1], f"o1{oc}")
            oo = small.tile([128, BPC], F32, tag=f"oo{oc}")
            nc.scalar.activation(oo, p_out1[oc], ACTF.Relu, scale=s6, bias=t6)
            o_o.append(oo)

        # ---- reg2 / att2 / out2 ----
        p_r2 = ps2.tile([7, NT], F32, tag="mm")
        nc.tensor.matmul(p_r2, W["w_reg2"], h_reg, start=True, stop=True)
        resid7 = small.tile([7, NT], F32, tag="resid7")
        nc.scalar.activation(resid7, p_r2, ACTF.Identity, bias=W["b_reg2"])
        p_rt = ps2.tile([NT, 7], F32, tag="mm")
        nc.tensor.transpose(p_rt, resid7, ident[0:7, 0:7])
        rT = small.tile([NT, 7], F32, tag="rT")
        nc.scalar.copy(rT, p_rt)

        p_a2 = ps2.tile([1, NT], F32, tag="mm")
        nc.tensor.matmul(p_a2, W["w_att2"], a_att, start=True, stop=True)
        logits = small.tile([1, NT], F32, tag="logits")
        nc.scalar.activation(logits, p_a2, ACTF.Identity, bias=W["b_att2"])
        lmax = small.tile([1, BPC], F32, tag="lmax")
        nc.vector.tensor_reduce(lmax,
                                logits.rearrange("q (b r) -> q b r", b=BPC),
                                axis=AX.X, op=ALU.max)
        lsh = small.tile([1, NT], F32, tag="lsh")
        nc.vector.tensor_tensor(
            lsh.rearrange("q (b r) -> q b r", b=BPC),
            logits.rearrange("q (b r) -> q b r", b=BPC),
            lmax.unsqueeze(2).broadcast_to([1, BPC, NROT]), op=ALU.subtract)
        lexp = small.tile([1, NT], F32, tag="lexp")
        nc.scalar.activation(lexp, lsh, ACTF.Exp)
        lsum = small.tile([1, BPC], F32, tag="lsum")
        nc.vector.tensor_reduce(lsum,
                                lexp.rearrange("q (b r) -> q b r", b=BPC),
                                axis=AX.X, op=ALU.add)
        lrec = small.tile([1, BPC], F32, tag="lrec")
        nc.vector.reciprocal(lrec, lsum)
        attn = small.tile([1, NT], F32, tag="attn")
        nc.vector.tensor_tensor(
            attn.rearrange("q (b r) -> q b r", b=BPC),
            lexp.rearrange("q (b r) -> q b r", b=BPC),
            lrec.unsqueeze(2).broadcast_to([1, BPC, NROT]), op=ALU.mult)

        wo2 = W["w_out2"].rearrange("q (k o) -> q k o", k=4)
        xout_sb = []
        for oc in range(2):
            po = ps2.tile([128, BPC], F32, tag="mm")
            for k in range(4):
                nc.tensor.matmul(po, wo2[:, k, oc * 128:(oc + 1) * 128],
                                 o_o[k], start=(k == 0), stop=(k == 3))
            xs = small.tile([128, BPC], F32, tag=f"xout{oc}")
            nc.scalar.activation(xs, po, ACTF.Identity,
                                 bias=W["b_out2"][:, oc:oc + 1])
            xout_sb.append(xs)

        # ---- SO(3) exp map + pred_R ----
        d3 = rT[:, 0:3]
        sq = small.tile([NT, 3], F32, tag="sq")
        nc.vector.tensor_tensor(sq, d3, d3, op=ALU.mult)
        nrm2 = small.tile([NT, 1], F32, tag="nrm2")
        nc.vector.tensor_reduce(nrm2, sq, axis=AX.X, op=ALU.add)
        nrm = small.tile([NT, 1], F32, tag="nrm")
        nc.scalar.activation(nrm, nrm2, ACTF.Sqrt)
        ninv = small.tile([NT, 1], F32, tag="ninv")
        nc.vector.reciprocal(ninv, nrm)
        D = small.tile([NT, 3], F32, tag="D")
        nc.vector.tensor_scalar_mul(D, d3, ninv)
        sg = small.tile([NT, 1], F32, tag="sg")
        nc.scalar.activation(sg, rT[:, 3:4], ACTF.Sigmoid)
        Nv = small.tile([NT, 1], F32, tag="Nv")
        nc.scalar.activation(Nv, sg, ACTF.Identity,
                             scale=np.pi / 5.0, bias=-np.pi / 10.0)
        th = small.tile([NT, 1], F32, tag="th")
        nc.scalar.activation(th, Nv, ACTF.Abs)
        sgn = small.tile([NT, 1], F32, tag="sgn")
        nc.scalar.sign(sgn, Nv)
        kv = small.tile([NT, 3], F32, tag="kv")
        nc.vector.tensor_scalar_mul(kv, D, sgn)
        sin_t = small.tile([NT, 1], F32, tag="sin_t")
        nc.scalar.activation(sin_t, th, ACTF.Sin)
        cos_t = small.tile([NT, 1], F32, tag="cos_t")
        nc.scalar.activation(cos_t, th, ACTF.Sin, bias=np.pi / 2.0)
        cm1 = small.tile([NT, 1], F32, tag="cm1")
        nc.vector.tensor_scalar(cm1, cos_t, scalar1=-1.0, scalar2=1.0,
                                op0=ALU.mult, op1=ALU.add)
        R9 = small.tile([NT, 9], F32, tag="R9")
        nc.vector.tensor_tensor(
            R9.rearrange("q (i j) -> q i j", i=3),
            kv.unsqueeze(2).broadcast_to([NT, 3, 3]),
            kv.unsqueeze(1).broadcast_to([NT, 3, 3]), op=ALU.mult)
        nc.vector.tensor_scalar_mul(R9, R9, cm1)
        sk = small.tile([NT, 3], F32, tag="sk")
        nc.vector.tensor_scalar_mul(sk, kv, sin_t)
        for (slot, comp, sign) in ((1, 2, -1), (2, 1, 1), (3, 2, 1),
                                   (5, 0, -1), (6, 1, -1), (7, 0, 1)):
            op = ALU.add if sign > 0 else ALU.subtract
            nc.vector.tensor_tensor(R9[:, slot:slot + 1],
                                    R9[:, slot:slot + 1],
                                    sk[:, comp:comp + 1], op=op)
        for d in range(3):
            nc.vector.tensor_tensor(R9[:, 4 * d:4 * d + 1],
                                    R9[:, 4 * d:4 * d + 1], cos_t,
                                    op=ALU.add)
        predR = small.tile([NT, 9], F32, tag="predR")
        tmp9 = small.tile([NT, 9], F32, tag="tmp9")
        A3 = W["anch"].rearrange("q (i j) -> q i j", i=3)
        R3 = R9.rearrange("q (i j) -> q i j", i=3)
        for j in range(3):
            a_ij = A3[:, :, j].unsqueeze(2).broadcast_to([NT, 3, 3])
            r_jk = R3[:, j, :].unsqueeze(1).broadcast_to([NT, 3, 3])
            if j == 0:
                nc.vector.tensor_tensor(
                    predR.rearrange("q (i k) -> q i k", i=3),
                    a_ij, r_jk, op=ALU.mult)
            else:
                nc.vector.tensor_tensor(
                    tmp9.rearrange("q (i k) -> q i k", i=3),
                    a_ij, r_jk, op=ALU.mult)
                nc.vector.tensor_tensor(predR, predR, tmp9, op=ALU.add)

        # ---- outputs ----
        for oc in range(2):
            nc.sync.dma_start(
                outs["x_out"].ap()[:, oc * 128:(oc + 1) * 128]
                .transpose([1, 0]), xout_sb[oc])
        nc.sync.dma_start(
            outs["x_attn"].ap().rearrange("b r -> (b r)").unsqueeze(0), attn)
        nc.sync.dma_start(
            outs["pred_R"].ap().rearrange("b r i j -> (b r) (i j)"), predR)
        nc.sync.dma_start(
            outs["res_T"].ap().rearrange("b r k -> (b r) k"), rT[:, 4:7])

        if "dbg_pool" in outs:
            dp = outs["dbg_pool"].ap()
            for b in range(BPC):
                nc.sync.dma_start(dp[b, 0:64, :], p0[b])
                nc.sync.dma_start(dp[b, 64:192, :], pools[("pn1", b)])
            nc.sync.dma_start(outs["dbg_xf"].ap(), xf[0])
            dl = outs["dbg_lin"].ap()
            for b in range(BPC):
                nc.sync.dma_start(dl[b], linT[b])


_NC_CACHE = {}


def _get_nc(debug_outs=False):
    key = bool(debug_outs)
    if key not in _NC_CACHE:
        _NC_CACHE[key] = _build(debug_outs=key)
    return _NC_CACHE[key]


def _stage_inputs(inputs):
    bf = ml_dtypes.bfloat16
    xyz0 = np.asarray(inputs["xyz0"], np.float32)
    feats0 = np.asarray(inputs["feats0"], np.float32)
    xyz1 = np.asarray(inputs["xyz1"], np.float32)
    feats1 = np.asarray(inputs["feats1"], np.float32)
    trace_idx = np.asarray(inputs["trace_idx"])
    anchors = np.asarray(inputs["anchors"], np.float32)

    def col(v):
        return np.ascontiguousarray(np.asarray(v, np.float32).reshape(-1, 1))

    def colk(v):
        a = np.asarray(v, np.float32)
        return np.ascontiguousarray(a.reshape(-1, 128).T)

    W0 = np.asarray(inputs["pn0_W"], np.float32)
    W1 = np.asarray(inputs["pn1_W"], np.float32)
    w_pn0f = np.zeros((128, 128), np.float32)
    w_pn0f[0:64, 0:64] = W0[:, 3:].T
    w_pn0f[64:128, 64:128] = W0[:, 3:].T
    w_pn0x = np.zeros((6, 128), np.float32)
    w_pn0x[0:3, 0:64] = W0[:, 0:3].T
    w_pn0x[3:6, 64:128] = W0[:, 0:3].T

    linW = np.asarray(inputs["lin_W"], np.float32)
    featW = np.asarray(inputs["feat_W"], np.float32)
    regW1 = np.asarray(inputs["reg_W1"], np.float32)
    attW1 = np.asarray(inputs["att_W1"], np.float32)
    outW1 = np.asarray(inputs["out_W1"], np.float32)
    outW2 = np.asarray(inputs["out_W2"], np.float32)

    shared = {
        "w_pn0f": w_pn0f.astype(bf), "w_pn0x": w_pn0x.astype(bf),
        "w_pn1f": np.ascontiguousarray(W1[:, 3:].T).astype(bf),
        "w_pn1x": np.ascontiguousarray(W1[:, 0:3].T).astype(bf),
        "g_pn0": col(inputs["pn0_g"]), "bb_pn0": col(inputs["pn0_bb"]),
        "g_pn1": col(inputs["pn1_g"]), "bb_pn1": col(inputs["pn1_bb"]),
        "w_lin0": np.ascontiguousarray(linW.T[0:64]),
        "w_lin1": np.ascontiguousarray(linW.T[64:192]),
        "g_lin": colk(inputs["lin_g"]), "bb_lin": colk(inputs["lin_bb"]),
        "ti": np.ascontiguousarray(
            trace_idx.astype(np.int32).reshape(1, NA * NROT)),
        "w_feat": np.ascontiguousarray(
            featW.reshape(512, 2, 128, NA).transpose(2, 1, 3, 0)
            .reshape(128, 2 * NA * 512)).astype(bf),
        "g_feat": colk(inputs["feat_g"]), "bb_feat": colk(inputs["feat_bb"]),
        "w_reg1": np.ascontiguousarray(
            regW1.T.reshape(4, 128, 128).transpose(1, 0, 2).reshape(128, 512)),
        "g_reg": col(inputs["reg_g"]), "bb_reg": col(inputs["reg_bb"]),
        "w_att1": np.ascontiguousarray(
            attW1.T.reshape(4, 128, 128).transpose(1, 0, 2).reshape(128, 512)),
        "g_att": col(inputs["att_g"]), "bb_att": col(inputs["att_bb"]),
        "w_reg2": np.ascontiguousarray(
            np.asarray(inputs["reg_W2"], np.float32).T),
        "b_reg2": col(inputs["reg_b2"]),
        "w_att2": np.ascontiguousarray(
            np.asarray(inputs["att_W2"], np.float32).T),
        "b_att2": col(inputs["att_b2"]),
        "w_out1": np.ascontiguousarray(
            outW1.T.reshape(4, 128, 512).transpose(1, 0, 2)
            .reshape(128, 2048)),
        "g_out": colk(inputs["out_g"]), "bb_out": colk(inputs["out_bb"]),
        "w_out2": np.ascontiguousarray(
            outW2.T.reshape(4, 128, 256).transpose(1, 0, 2)
            .reshape(128, 1024)),
        "b_out2": colk(inputs["out_b2"]),
        "anch": np.ascontiguousarray(
            np.tile(anchors.reshape(NROT, 9), (BPC, 1))),
    }

    in_maps = []
    for r in range(N_CORES):
        b0 = r * BPC
        m = dict(shared)
        # f0: (b, (h,c), (p',a)) point-half packing
        f0s = feats0[b0:b0 + BPC].reshape(BPC, C0, 2, NP // 2, NA) \
            .transpose(0, 2, 1, 3, 4).reshape(BPC, 128, 6144)
        m["f0"] = np.ascontiguousarray(f0s).astype(bf)
        m["f1"] = np.ascontiguousarray(
            feats1[b0:b0 + BPC].reshape(BPC, C1, NP * NA)).astype(bf)
        x0s = xyz0[b0:b0 + BPC].reshape(BPC, 3, 2, NP // 2) \
            .transpose(0, 2, 1, 3).reshape(BPC, 6, NP // 2)
        m["x0"] = np.ascontiguousarray(x0s).astype(bf)
        m["x1"] = np.ascontiguousarray(xyz1[b0:b0 + BPC]).astype(bf)
        in_maps.append(m)
    return in_maps


def _run(inputs, trace=False, debug_outs=False):
    nc = _get_nc(debug_outs=debug_outs)
    in_maps = _stage_inputs(inputs)
    res = bass_utils.run_bass_kernel_spmd(
        nc, in_maps, core_ids=list(range(N_CORES)), trace=trace)
    cat = lambda k: np.concatenate(  # noqa: E731
        [res.results[r][k] for r in range(N_CORES)])
    out = (cat("x_out").astype(np.float32), cat("x_attn").astype(np.float32),
           cat("pred_R").astype(np.float32), cat("res_T").astype(np.float32))
    return out, res


def kernel(**inputs):
    out, _ = _run(inputs, trace=False)
    return out


# revision 8
# speedup vs baseline: 1.0572x; 1.0572x over previous
# Distributed Trainium2 (8 NeuronCore) Bass kernel for nn_ArtOutBlock.
#
# Sharding: data-parallel over batch (16 batches -> 2 per core) for the heavy
# pointnet conv + max-pool phase; exact BatchNorm batch statistics via three
# small AllGathers of per-core partial bn_stats (+ pooled values), aggregated
# on-device with bn_aggr.
import os
import sys
import types

sys.path.insert(0, "/opt/trn_rl_repo")

import numpy as np
import ml_dtypes

# --- NTFF profile hook (antenv.axon_hooks is stubbed out in this image) ----
import antenv  # noqa: E402

if "antenv.axon_hooks" not in sys.modules:
    _m = types.ModuleType("antenv.axon_hooks")
    _m._hook = None
    _m.set_axon_ntff_profile_hook = lambda h: setattr(_m, "_hook", h)
    _m.get_axon_ntff_profile_hook = lambda: _m._hook
    sys.modules["antenv.axon_hooks"] = _m
    antenv.axon_hooks = _m

try:
    from trn_agent_boot.trn_boot import _ntff_profile_via_ctypes

    sys.modules["antenv.axon_hooks"].set_axon_ntff_profile_hook(
        _ntff_profile_via_ctypes("/opt/axon/libaxon_pjrt.so")
    )
except Exception:
    pass

import concourse.bass as bass  # noqa: E402,F401
import concourse.bacc as bacc  # noqa: E402
import concourse.mybir as mybir  # noqa: E402
import concourse.tile as tile  # noqa: E402
from concourse import bass_utils  # noqa: E402
from concourse.masks import make_identity  # noqa: E402

bass_utils.upload_artifacts = lambda tmpdir: f"file://{tmpdir}"

F32 = mybir.dt.float32
BF16 = mybir.dt.bfloat16
I32 = mybir.dt.int32
AX = mybir.AxisListType
ALU = mybir.AluOpType
ACTF = mybir.ActivationFunctionType

N_CORES = 8
NB, NP, NA, NROT = 16, 1024, 12, 60
C0, C1 = 64, 128
CIN, CFEAT, COUT = 256, 512, 256
BPC = NB // N_CORES  # batches per core = 2
NT = BPC * NROT      # 120
EPS = 1e-5

# bounce1 per-rank layout (f32 words)
B1_P0 = 0            # pn0 pooled, per b: 64*12      -> 2*768
B1_P1 = 1536         # pn1 pooled, per b: 128*12     -> 2*1536
B1_S0 = 4608         # pn0 bn_stats, per b: 64*6     -> 2*384
B1_S1 = 5376         # pn1 bn_stats, per b: 128*6    -> 2*768
B1_N = 6912
B2_N = 4 * 128 * 6
B3_N = 6 * 128 * 6

WEIGHT_SPECS = [
    ("w_pn0f", (128, 128), BF16), ("w_pn0x", (6, 128), BF16),
    ("w_pn1f", (128, 128), BF16), ("w_pn1x", (3, 128), BF16),
    ("g_pn0", (64, 1), F32), ("bb_pn0", (64, 1), F32),
    ("g_pn1", (128, 1), F32), ("bb_pn1", (128, 1), F32),
    ("w_lin0", (64, 256), F32), ("w_lin1", (128, 256), F32),
    ("g_lin", (128, 2), F32), ("bb_lin", (128, 2), F32),
    ("ti", (1, 720), I32),
    ("w_feat", (128, 2 * NA * CFEAT), BF16),
    ("g_feat", (128, 4), F32), ("bb_feat", (128, 4), F32),
    ("w_reg1", (128, 512), F32),
    ("g_reg", (128, 1), F32), ("bb_reg", (128, 1), F32),
    ("w_att1", (128, 512), F32),
    ("g_att", (128, 1), F32), ("bb_att", (128, 1), F32),
    ("w_reg2", (128, 7), F32), ("b_reg2", (7, 1), F32),
    ("w_att2", (128, 1), F32), ("b_att2", (1, 1), F32),
    ("w_out1", (128, 2048), F32),
    ("g_out", (128, 4), F32), ("bb_out", (128, 4), F32),
    ("w_out2", (128, 1024), F32), ("b_out2", (128, 2), F32),
    ("anch", (NT, 9), F32),
]


def _build(debug_outs=False):
    nc = bacc.Bacc("TRN2", target_bir_lowering=False, debug=False,
                   num_devices=N_CORES)
    params = {}

    def P(name, shape, dt):
        params[name] = nc.declare_dram_parameter(name, list(shape), dt,
                                                 isOutput=False)

    P("f0", (BPC, 128, 6144), BF16)   # (b, (h,c), (p',a))
    P("f1", (BPC, 128, NP * NA), BF16)
    P("x0", (BPC, 6, 6144), BF16)   # (b, (h,c), (p',a)) pre-broadcast
    P("x1", (BPC, 3, NP * NA), BF16)
    for name, shape, dt in WEIGHT_SPECS:
        P(name, shape, dt)

    outs = {
        "x_out": nc.declare_dram_parameter("x_out", [BPC, COUT], F32, True),
        "x_attn": nc.declare_dram_parameter("x_attn", [BPC, NROT], F32, True),
        "pred_R": nc.declare_dram_parameter("pred_R", [BPC, NROT, 3, 3], F32,
                                            True),
        "res_T": nc.declare_dram_parameter("res_T", [BPC, NROT, 3], F32, True),
    }
    if debug_outs:
        outs["dbg_pool"] = nc.declare_dram_parameter(
            "dbg_pool", [BPC, 192, NA], F32, True)
        outs["dbg_xf"] = nc.declare_dram_parameter(
            "dbg_xf", [128, NT], F32, True)
        outs["dbg_lin"] = nc.declare_dram_parameter(
            "dbg_lin", [BPC, NA, 256], F32, True)

    with tile.TileContext(nc) as tc:
        _graph(nc, tc, params, outs)
    nc.finalize()
    return nc


def _graph(nc, tc, prm, outs):
    from contextlib import ExitStack

    RG = [list(range(N_CORES))]
    ctx = ExitStack()
    with ctx:
        consts = ctx.enter_context(tc.tile_pool(name="consts", bufs=1))
        wpool = ctx.enter_context(tc.tile_pool(name="wpool", bufs=1))
        small = ctx.enter_context(tc.tile_pool(name="small", bufs=1))
        dram = ctx.enter_context(tc.tile_pool(name="dram", bufs=1,
                                              space="DRAM"))

        ident = consts.tile([128, 128], F32, tag="ident")
        make_identity(nc, ident)

        # register float constants used as activation biases
        for cv in (EPS, float(np.pi / 2.0), float(-np.pi / 10.0)):
            ct = consts.tile([128, 1], F32, tag=f"cst{cv}")
            nc.gpsimd.memset(ct, cv)
            nc.const_aps.aps[(F32, cv)] = ct

        PH1 = {"w_pn0f", "w_pn0x", "w_pn1f", "w_pn1x",
               "g_pn0", "bb_pn0", "g_pn1", "bb_pn1"}
        W = {}
        for name, shape, dt in WEIGHT_SPECS:
            if name in PH1:
                tl = wpool.tile(list(shape), dt, tag=name, name=f"w_{name}")
                nc.sync.dma_start(tl, prm[name].ap())
                W[name] = tl


        # ================= phase 1: convs + max pool =================
        bigin = ctx.enter_context(tc.tile_pool(name="bigin", bufs=2))
        xin = ctx.enter_context(tc.tile_pool(name="xin", bufs=2))
        pools = {}

        def conv_level(kind, b):
            if kind == "pn0":
                src = prm["f0"].ap()[b]      # (128, 6144)
                xsrc = prm["x0"].ap()[b]
                ncols, nslots = 6144, 4
                wf, wx = W["w_pn0f"], W["w_pn0x"]
                xt = xin.tile([6, 6144], BF16, tag="xin", name=f"x0_{b}")
            else:
                src = prm["f1"].ap()[b]      # (128, 12288)
                xsrc = prm["x1"].ap()[b]
                ncols, nslots = NP * NA, 8
                wf, wx = W["w_pn1f"], W["w_pn1x"]
                xt = xin.tile([3, NP * NA], BF16, tag="xin", name=f"x1_{b}")
            nc.sync.dma_start(xt, xsrc)

            inp = bigin.tile([128, ncols], BF16, tag="conv_in")
            nc.sync.dma_start(inp, src)
            slots = small.tile([128, nslots * NA], F32,
                               tag=f"slots_{kind}_{b}")
            with tc.tile_pool(name=f"cps_{kind}_{b}", bufs=2,
                              space="PSUM") as cpp:
                for s in range(nslots):  # each slot: 128 points
                    ps = cpp.tile([128, 2048], F32, tag="cpsum")
                    for m in range(4):   # 4 matmuls x 32 points
                        pbase = s * 128 + m * 32
                        nc.tensor.matmul(ps[:, m * 512:m * 512 + 384],
                                         wf, inp[:, pbase * NA:(pbase + 32) * NA],
                                         start=True, stop=False)
                        nc.tensor.matmul(
                            ps[:, m * 512:m * 512 + 384], wx,
                            xt[:, pbase * NA:(pbase + 32) * NA],
                            start=False, stop=True)
                    red = ps.rearrange("q (m c) -> q m c", m=4)[:, :, 0:384] \
                        .rearrange("q m (p a) -> q m p a", a=NA) \
                        .transpose([0, 3, 1, 2])  # (128, 12, 4, 32)
                    nc.vector.tensor_reduce(slots[:, s * NA:(s + 1) * NA],
                                            red, axis=AX.XY, op=ALU.max)
            pooled = small.tile([128, NA], F32, tag=f"pool_{kind}_{b}")
            nc.vector.tensor_reduce(
                pooled,
                slots.rearrange("q (s a) -> q s a", a=NA).transpose([0, 2, 1]),
                axis=AX.X, op=ALU.max)
            return pooled

        for b in range(BPC):
            pools[("pn1", b)] = conv_level("pn1", b)
        for b in range(BPC):
            pools[("pn0", b)] = conv_level("pn0", b)

        # phase-2 weights: issued after conv loads, on the scalar HWDGE queue
        for name, shape, dt in WEIGHT_SPECS:
            if name not in PH1:
                tl = wpool.tile(list(shape), dt, tag=name, name=f"w_{name}")
                nc.scalar.dma_start(tl, prm[name].ap())
                W[name] = tl

        # pn0: combine point-halves (partitions (h,o)) via transpose
        p0 = {}
        with tc.tile_pool(name="tps", bufs=2, space="PSUM") as tps:
            for b in range(BPC):
                pt = tps.tile([NA, 128], F32, tag="tp")
                nc.tensor.transpose(pt, pools[("pn0", b)], ident)
                pts = small.tile([NA, 128], F32, tag=f"pts{b}")
                nc.scalar.copy(pts, pt)
                hm = small.tile([NA, 64], F32, tag=f"hmax{b}")
                nc.vector.tensor_tensor(hm, pts[:, 0:64], pts[:, 64:128],
                                        op=ALU.max)
                bt = tps.tile([64, NA], F32, tag="tpb")
                nc.tensor.transpose(bt, hm, ident[0:NA, 0:NA])
                p0b = small.tile([64, NA], F32, tag=f"p0_{b}")
                nc.scalar.copy(p0b, bt)
                p0[b] = p0b

        stats1 = {}
        for b in range(BPC):
            s0 = small.tile([64, 6], F32, tag=f"s0_{b}")
            nc.vector.bn_stats(s0, p0[b])
            stats1[("pn0", b)] = s0
            s1 = small.tile([128, 6], F32, tag=f"s1_{b}")
            nc.vector.bn_stats(s1, pools[("pn1", b)])
            stats1[("pn1", b)] = s1

        # ---- bounce1 + AllGather 1 ----
        b1in = dram.tile([B1_N], F32, tag="b1in")
        b1out = dram.tile([N_CORES, B1_N], F32, tag="b1out")
        for b in range(BPC):
            nc.sync.dma_start(
                b1in[B1_P0 + b * 768:B1_P0 + (b + 1) * 768]
                .rearrange("(c a) -> c a", a=NA), p0[b])
            nc.sync.dma_start(
                b1in[B1_P1 + b * 1536:B1_P1 + (b + 1) * 1536]
                .rearrange("(c a) -> c a", a=NA), pools[("pn1", b)])
            nc.sync.dma_start(
                b1in[B1_S0 + b * 384:B1_S0 + (b + 1) * 384]
                .rearrange("(c k) -> c k", k=6), stats1[("pn0", b)])
            nc.sync.dma_start(
                b1in[B1_S1 + b * 768:B1_S1 + (b + 1) * 768]
                .rearrange("(c k) -> c k", k=6), stats1[("pn1", b)])
        nc.gpsimd.collective_compute(
            "AllGather", ALU.bypass, replica_groups=RG,
            ins=[b1in.opt()], outs=[b1out.opt()])
        g1 = b1out  # (8, B1_N)

        def agg_from(srcs, parts, tag):
            # srcs: list of 3-dim (parts, 8, 6) dram views
            st = small.tile([parts, len(srcs), 8, 6], F32, tag=f"aggin_{tag}",
                            name=f"aggin_{tag}")
            for i, sv in enumerate(srcs):
                nc.sync.dma_start(st[:, i], sv)
            ag = small.tile([parts, 2], F32, tag=f"agg_{tag}",
                            name=f"agg_{tag}")
            nc.vector.bn_aggr(ag, st)
            return ag

        def stat_view(off, nchan, b):
            return g1[:, off + b * nchan * 6:off + (b + 1) * nchan * 6] \
                .rearrange("r (c k) -> r c k", k=6).transpose([1, 0, 2])

        agg_pn0 = agg_from([stat_view(B1_S0, 64, b) for b in range(2)],
                           64, "pn0")
        agg_pn1 = agg_from([stat_view(B1_S1, 128, b) for b in range(2)],
                           128, "pn1")

        def mk_scale_shift(agg, g_t, bb_t, parts, tag):
            sd = small.tile([parts, 1], F32, tag=f"sd_{tag}")
            nc.scalar.activation(sd, agg[:, 1:2], ACTF.Sqrt, bias=EPS)
            rs = small.tile([parts, 1], F32, tag=f"rs_{tag}")
            nc.vector.reciprocal(rs, sd)
            s = small.tile([parts, 1], F32, tag=f"s_{tag}")
            nc.vector.tensor_tensor(s, rs, g_t, op=ALU.mult)
            ms = small.tile([parts, 1], F32, tag=f"ms_{tag}")
            nc.vector.tensor_tensor(ms, agg[:, 0:1], s, op=ALU.mult)
            sh = small.tile([parts, 1], F32, tag=f"sh_{tag}")
            nc.vector.tensor_tensor(sh, bb_t, ms, op=ALU.subtract)
            return s, sh

        s_pn0, t_pn0 = mk_scale_shift(agg_pn0, W["g_pn0"], W["bb_pn0"],
                                      64, "pn0")
        s_pn1, t_pn1 = mk_scale_shift(agg_pn1, W["g_pn1"], W["bb_pn1"],
                                      128, "pn1")

        # ---- all-batch pooled -> lin stats ----
        tileA = small.tile([64, 2, 8, NA], F32, tag="tileA")
        tileB = small.tile([128, 2, 8, NA], F32, tag="tileB")
        for b in range(2):
            nc.sync.dma_start(
                tileA[:, b],
                g1[:, B1_P0 + b * 768:B1_P0 + (b + 1) * 768]
                .rearrange("r (c a) -> r c a", a=NA).transpose([1, 0, 2]))
            nc.sync.dma_start(
                tileB[:, b],
                g1[:, B1_P1 + b * 1536:B1_P1 + (b + 1) * 1536]
                .rearrange("r (c a) -> r c a", a=NA).transpose([1, 0, 2]))
        nc.scalar.activation(tileA, tileA, ACTF.Relu, scale=s_pn0, bias=t_pn0)
        nc.scalar.activation(tileB, tileB, ACTF.Relu, scale=s_pn1, bias=t_pn1)

        ps2 = ctx.enter_context(tc.tile_pool(name="ps2", bufs=4, space="PSUM"))
        psf = ctx.enter_context(tc.tile_pool(name="psf", bufs=4, space="PSUM"))

        lin_s, lin_t = [], []
        for oc in range(2):
            pl = ps2.tile([128, 192], F32, tag="mm")
            nc.tensor.matmul(pl, W["w_lin0"][:, oc * 128:(oc + 1) * 128],
                             tileA.rearrange("c b r a -> c (b r a)"),
                             start=True, stop=False)
            nc.tensor.matmul(pl, W["w_lin1"][:, oc * 128:(oc + 1) * 128],
                             tileB.rearrange("c b r a -> c (b r a)"),
                             start=False, stop=True)
            st = small.tile([128, 6], F32, tag=f"linst{oc}")
            nc.vector.bn_stats(st, pl)
            ag = small.tile([128, 2], F32, tag=f"linag{oc}")
            nc.vector.bn_aggr(ag, st)
            s2, t2 = mk_scale_shift(ag, W["g_lin"][:, oc:oc + 1],
                                    W["bb_lin"][:, oc:oc + 1],
                                    128, f"lin{oc}")
            lin_s.append(s2)
            lin_t.append(t2)

        # ---- own-batch pooled BN -> lin -> transpose ----
        p0bn, p1bn = {}, {}
        for b in range(BPC):
            a0 = small.tile([64, NA], F32, tag=f"p0bn{b}")
            nc.scalar.activation(a0, p0[b], ACTF.Relu, scale=s_pn0, bias=t_pn0)
            p0bn[b] = a0
            a1 = small.tile([128, NA], F32, tag=f"p1bn{b}")
            nc.scalar.activation(a1, pools[("pn1", b)], ACTF.Relu,
                                 scale=s_pn1, bias=t_pn1)
            p1bn[b] = a1

        linT = {b: small.tile([NA, 256], BF16, tag=f"linT{b}",
                               name=f"linT{b}") for b in range(BPC)}
        lin_bn_dbg = {}
        for b in range(BPC):
            for oc in range(2):
                pl = ps2.tile([128, NA], F32, tag="mm")
                nc.tensor.matmul(pl, W["w_lin0"][:, oc * 128:(oc + 1) * 128],
                                 p0bn[b], start=True, stop=False)
                nc.tensor.matmul(pl, W["w_lin1"][:, oc * 128:(oc + 1) * 128],
                                 p1bn[b], start=False, stop=True)
                lb = small.tile([128, NA], F32, tag=f"linbn{b}{oc}")
                nc.scalar.activation(lb, pl, ACTF.Identity,
                                     scale=lin_s[oc], bias=lin_t[oc])
                lin_bn_dbg[(b, oc)] = lb
                pt = ps2.tile([NA, 128], F32, tag="mm")
                nc.tensor.transpose(pt, lb, ident)
                nc.scalar.copy(linT[b][:, oc * 128:(oc + 1) * 128], pt)

        # ---- one-hot from trace_idx ----
        ti_f1 = small.tile([1, 720], F32, tag="ti_f1")
        nc.vector.tensor_copy(ti_f1, W["ti"])
        ti_f = small.tile([NA, 720], F32, tag="ti_f")
        nc.gpsimd.partition_broadcast(ti_f, ti_f1)
        io_i = small.tile([NA, 720], I32, tag="io_i")
        nc.gpsimd.iota(io_i, pattern=[[0, 720]], base=0, channel_multiplier=1)
        io_f = small.tile([NA, 720], F32, tag="io_f")
        nc.vector.tensor_copy(io_f, io_i)
        oh = small.tile([NA, 720], BF16, tag="oh")
        nc.vector.tensor_tensor(oh, ti_f, io_f, op=ALU.is_equal)

        # ---- gather matmuls -> X2 ----
        X2 = [small.tile([128, BPC * 720], BF16, tag=f"X2_{cc}",
                         name=f"X2_{cc}") for cc in range(2)]
        for b in range(BPC):
            for cc in range(2):
                for j in range(2):
                    pg = ps2.tile([128, 360], F32, tag="mm")
                    nc.tensor.matmul(pg, linT[b][:, cc * 128:(cc + 1) * 128],
                                     oh[:, j * 360:(j + 1) * 360],
                                     start=True, stop=True)
                    nc.scalar.copy(
                        X2[cc][:, b * 720 + j * 360:b * 720 + (j + 1) * 360],
                        pg)

        # ---- feat matmuls ----
        wf_v = W["w_feat"].rearrange("q (cc a o) -> q cc a o", cc=2, a=NA)
        feat_ps = []
        for oc in range(4):
            pf = psf.tile([128, NT], F32, tag="featmm")
            first = True
            for cc in range(2):
                for a in range(NA):
                    rhs = X2[cc].rearrange("q (b a r) -> q b a r",
                                           b=BPC, a=NA)[:, :, a, :]
                    nc.tensor.matmul(pf,
                                     wf_v[:, cc, a, oc * 128:(oc + 1) * 128],
                                     rhs, start=first,
                                     stop=(cc == 1 and a == NA - 1))
                    first = False
            feat_ps.append(pf)

        # ---- feat stats -> AllGather 2 ----
        b2in = dram.tile([B2_N], F32, tag="b2in")
        b2out = dram.tile([N_CORES, B2_N], F32, tag="b2out")
        for oc in range(4):
            fs = small.tile([128, 6], F32, tag=f"fstat{oc}")
            nc.vector.bn_stats(fs, feat_ps[oc])
            nc.sync.dma_start(
                b2in[oc * 768:(oc + 1) * 768].rearrange("(c k) -> c k", k=6),
                fs)
        nc.gpsimd.collective_compute(
            "AllGather", ALU.bypass, replica_groups=RG,
            ins=[b2in.opt()], outs=[b2out.opt()])

        xf, xm = [], []
        for oc in range(4):
            ag = agg_from(
                [b2out[:, oc * 768:(oc + 1) * 768]
                 .rearrange("r (c k) -> r c k", k=6).transpose([1, 0, 2])],
                128, f"feat{oc}")
            s3, t3 = mk_scale_shift(ag, W["g_feat"][:, oc:oc + 1],
                                    W["bb_feat"][:, oc:oc + 1],
                                    128, f"feat{oc}")
            x = small.tile([128, NT], F32, tag=f"xf{oc}")
            nc.scalar.activation(x, feat_ps[oc], ACTF.Relu, scale=s3, bias=t3)
            xf.append(x)
            m = small.tile([128, BPC], F32, tag=f"xm{oc}")
            nc.vector.tensor_reduce(m, x.rearrange("q (b r) -> q b r", b=BPC),
                                    axis=AX.X, op=ALU.max)
            xm.append(m)

        # ---- reg1 / att1 / out1 ----
        wr1 = W["w_reg1"].rearrange("q (k o) -> q k o", k=4)
        wa1 = W["w_att1"].rearrange("q (k o) -> q k o", k=4)
        wo1 = W["w_out1"].rearrange("q (k o) -> q k o", k=4)
        p_reg = ps2.tile([128, NT], F32, tag="mm")
        p_att = ps2.tile([128, NT], F32, tag="mm")
        for k in range(4):
            nc.tensor.matmul(p_reg, wr1[:, k, :], xf[k],
                             start=(k == 0), stop=(k == 3))
        for k in range(4):
            nc.tensor.matmul(p_att, wa1[:, k, :], xf[k],
                             start=(k == 0), stop=(k == 3))
        p_out1 = []
        for oc in range(4):
            po = psf.tile([128, BPC], F32, tag="featmm")
            for k in range(4):
                nc.tensor.matmul(po, wo1[:, k, oc * 128:(oc + 1) * 128],
                                 xm[k], start=(k == 0), stop=(k == 3))
            p_out1.append(po)

        # ---- stats -> AllGather 3 ----
        b3in = dram.tile([B3_N], F32, tag="b3in")
        b3out = dram.tile([N_CORES, B3_N], F32, tag="b3out")

        def put_stats(ps_t, off, tag):
            st = small.tile([128, 6], F32, tag=f"st3_{tag}")
            nc.vector.bn_stats(st, ps_t)
            nc.sync.dma_start(
                b3in[off:off + 768].rearrange("(c k) -> c k", k=6), st)

        put_stats(p_reg, 0, "reg")
        put_stats(p_att, 768, "att")
        for oc in range(4):
            put_stats(p_out1[oc], 1536 + oc * 768, f"o1{oc}")
        nc.gpsimd.collective_compute(
            "AllGather", ALU.bypass, replica_groups=RG,
            ins=[b3in.opt()], outs=[b3out.opt()])

        def agg3(off, g_t, bb_t, tag):
            ag = agg_from(
                [b3out[:, off:off + 768]
                 .rearrange("r (c k) -> r c k", k=6).transpose([1, 0, 2])],
                128, f"a3_{tag}")
            return mk_scale_shift(ag, g_t, bb_t, 128, f"h_{tag}")

        s4, t4 = agg3(0, W["g_reg"], W["bb_reg"], "reg")
        s5, t5 = agg3(768, W["g_att"], W["bb_att"], "att")
        h_reg = small.tile([128, NT], F32, tag="h_reg")
        nc.scalar.activation(h_reg, p_reg, ACTF.Relu, scale=s4, bias=t4)
        a_att = small.tile([128, NT], F32, tag="a_att")
        nc.scalar.activation(a_att, p_att, ACTF.Relu, scale=s5, bias=t5)
        o_o = []
        for oc in range(4):
            s6, t6 = agg3(1536 + oc * 768, W["g_out"][:, oc:oc + 1],
                          W["bb_out"][:, oc:oc + 1], f"o1{oc}")
            oo = small.tile([128, BPC], F32, tag=f"oo{oc}")
            nc.scalar.activation(oo, p_out1[oc], ACTF.Relu, scale=s6, bias=t6)
            o_o.append(oo)

        # ---- reg2 / att2 / out2 ----
        p_r2 = ps2.tile([7, NT], F32, tag="mm")
        nc.tensor.matmul(p_r2, W["w_reg2"], h_reg, start=True, stop=True)
        resid7 = small.tile([7, NT], F32, tag="resid7")
        nc.scalar.activation(resid7, p_r2, ACTF.Identity, bias=W["b_reg2"])
        p_rt = ps2.tile([NT, 7], F32, tag="mm")
        nc.tensor.transpose(p_rt, resid7, ident[0:7, 0:7])
        rT = small.tile([NT, 7], F32, tag="rT")
        nc.scalar.copy(rT, p_rt)

        p_a2 = ps2.tile([1, NT], F32, tag="mm")
        nc.tensor.matmul(p_a2, W["w_att2"], a_att, start=True, stop=True)
        logits = small.tile([1, NT], F32, tag="logits")
        nc.scalar.activation(logits, p_a2, ACTF.Identity, bias=W["b_att2"])
        lmax = small.tile([1, BPC], F32, tag="lmax")
        nc.vector.tensor_reduce(lmax,
                                logits.rearrange("q (b r) -> q b r", b=BPC),
                                axis=AX.X, op=ALU.max)
        lsh = small.tile([1, NT], F32, tag="lsh")
        nc.vector.tensor_tensor(
            lsh.rearrange("q (b r) -> q b r", b=BPC),
            logits.rearrange("q (b r) -> q b r", b=BPC),
            lmax.unsqueeze(2).broadcast_to([1, BPC, NROT]), op=ALU.subtract)
        lexp = small.tile([1, NT], F32, tag="lexp")
        nc.scalar.activation(lexp, lsh, ACTF.Exp)
        lsum = small.tile([1, BPC], F32, tag="lsum")
        nc.vector.tensor_reduce(lsum,
                                lexp.rearrange("q (b r) -> q b r", b=BPC),
                                axis=AX.X, op=ALU.add)
        lrec = small.tile([1, BPC], F32, tag="lrec")
        nc.vector.reciprocal(lrec, lsum)
        attn = small.tile([1, NT], F32, tag="attn")
        nc.vector.tensor_tensor(
            attn.rearrange("q (b r) -> q b r", b=BPC),
            lexp.rearrange("q (b r) -> q b r", b=BPC),
            lrec.unsqueeze(2).broadcast_to([1, BPC, NROT]), op=ALU.mult)

        wo2 = W["w_out2"].rearrange("q (k o) -> q k o", k=4)
        xout_sb = []
        for oc in range(2):
            po = ps2.tile([128, BPC], F32, tag="mm")
            for k in range(4):
                nc.tensor.matmul(po, wo2[:, k, oc * 128:(oc + 1) * 128],
                                 o_o[k], start=(k == 0), stop=(k == 3))
            xs = small.tile([128, BPC], F32, tag=f"xout{oc}")
            nc.scalar.activation(xs, po, ACTF.Identity,
                                 bias=W["b_out2"][:, oc:oc + 1])
            xout_sb.append(xs)

        # ---- SO(3) exp map + pred_R ----
        d3 = rT[:, 0:3]
        sq = small.tile([NT, 3], F32, tag="sq")
        nc.vector.tensor_tensor(sq, d3, d3, op=ALU.mult)
        nrm2 = small.tile([NT, 1], F32, tag="nrm2")
        nc.vector.tensor_reduce(nrm2, sq, axis=AX.X, op=ALU.add)
        nrm = small.tile([NT, 1], F32, tag="nrm")
        nc.scalar.activation(nrm, nrm2, ACTF.Sqrt)
        ninv = small.tile([NT, 1], F32, tag="ninv")
        nc.vector.reciprocal(ninv, nrm)
        D = small.tile([NT, 3], F32, tag="D")
        nc.vector.tensor_scalar_mul(D, d3, ninv)
        sg = small.tile([NT, 1], F32, tag="sg")
        nc.scalar.activation(sg, rT[:, 3:4], ACTF.Sigmoid)
        Nv = small.tile([NT, 1], F32, tag="Nv")
        nc.scalar.activation(Nv, sg, ACTF.Identity,
                             scale=np.pi / 5.0, bias=-np.pi / 10.0)
        th = small.tile([NT, 1], F32, tag="th")
        nc.scalar.activation(th, Nv, ACTF.Abs)
        sgn = small.tile([NT, 1], F32, tag="sgn")
        nc.scalar.sign(sgn, Nv)
        kv = small.tile([NT, 3], F32, tag="kv")
        nc.vector.tensor_scalar_mul(kv, D, sgn)
        sin_t = small.tile([NT, 1], F32, tag="sin_t")
        nc.scalar.activation(sin_t, th, ACTF.Sin)
        cos_t = small.tile([NT, 1], F32, tag="cos_t")
        nc.scalar.activation(cos_t, th, ACTF.Sin, bias=np.pi / 2.0)
        cm1 = small.tile([NT, 1], F32, tag="cm1")
        nc.vector.tensor_scalar(cm1, cos_t, scalar1=-1.0, scalar2=1.0,
                                op0=ALU.mult, op1=ALU.add)
        R9 = small.tile([NT, 9], F32, tag="R9")
        nc.vector.tensor_tensor(
            R9.rearrange("q (i j) -> q i j", i=3),
            kv.unsqueeze(2).broadcast_to([NT, 3, 3]),
            kv.unsqueeze(1).broadcast_to([NT, 3, 3]), op=ALU.mult)
        nc.vector.tensor_scalar_mul(R9, R9, cm1)
        sk = small.tile([NT, 3], F32, tag="sk")
        nc.vector.tensor_scalar_mul(sk, kv, sin_t)
        for (slot, comp, sign) in ((1, 2, -1), (2, 1, 1), (3, 2, 1),
                                   (5, 0, -1), (6, 1, -1), (7, 0, 1)):
            op = ALU.add if sign > 0 else ALU.subtract
            nc.vector.tensor_tensor(R9[:, slot:slot + 1],
                                    R9[:, slot:slot + 1],
                                    sk[:, comp:comp + 1], op=op)
        for d in range(3):
            nc.vector.tensor_tensor(R9[:, 4 * d:4 * d + 1],
                                    R9[:, 4 * d:4 * d + 1], cos_t,
                                    op=ALU.add)
        predR = small.tile([NT, 9], F32, tag="predR")
        tmp9 = small.tile([NT, 9], F32, tag="tmp9")
        A3 = W["anch"].rearrange("q (i j) -> q i j", i=3)
        R3 = R9.rearrange("q (i j) -> q i j", i=3)
        for j in range(3):
            a_ij = A3[:, :, j].unsqueeze(2).broadcast_to([NT, 3, 3])
            r_jk = R3[:, j, :].unsqueeze(1).broadcast_to([NT, 3, 3])
            if j == 0:
                nc.vector.tensor_tensor(
                    predR.rearrange("q (i k) -> q i k", i=3),
                    a_ij, r_jk, op=ALU.mult)
            else:
                nc.vector.tensor_tensor(
                    tmp9.rearrange("q (i k) -> q i k", i=3),
                    a_ij, r_jk, op=ALU.mult)
                nc.vector.tensor_tensor(predR, predR, tmp9, op=ALU.add)

        # ---- outputs ----
        for oc in range(2):
            nc.sync.dma_start(
                outs["x_out"].ap()[:, oc * 128:(oc + 1) * 128]
                .transpose([1, 0]), xout_sb[oc])
        nc.sync.dma_start(
            outs["x_attn"].ap().rearrange("b r -> (b r)").unsqueeze(0), attn)
        nc.sync.dma_start(
            outs["pred_R"].ap().rearrange("b r i j -> (b r) (i j)"), predR)
        nc.sync.dma_start(
            outs["res_T"].ap().rearrange("b r k -> (b r) k"), rT[:, 4:7])

        if "dbg_pool" in outs:
            dp = outs["dbg_pool"].ap()
            for b in range(BPC):
                nc.sync.dma_start(dp[b, 0:64, :], p0[b])
                nc.sync.dma_start(dp[b, 64:192, :], pools[("pn1", b)])
            nc.sync.dma_start(outs["dbg_xf"].ap(), xf[0])
            dl = outs["dbg_lin"].ap()
            for b in range(BPC):
                nc.sync.dma_start(dl[b], linT[b])


_NC_CACHE = {}


def _get_nc(debug_outs=False):
    key = bool(debug_outs)
    if key not in _NC_CACHE:
        _NC_CACHE[key] = _build(debug_outs=key)
    return _NC_CACHE[key]


def _stage_inputs(inputs):
    bf = ml_dtypes.bfloat16
    xyz0 = np.asarray(inputs["xyz0"], np.float32)
    feats0 = np.asarray(inputs["feats0"], np.float32)
    xyz1 = np.asarray(inputs["xyz1"], np.float32)
    feats1 = np.asarray(inputs["feats1"], np.float32)
    trace_idx = np.asarray(inputs["trace_idx"])
    anchors = np.asarray(inputs["anchors"], np.float32)

    def col(v):
        return np.ascontiguousarray(np.asarray(v, np.float32).reshape(-1, 1))

    def colk(v):
        a = np.asarray(v, np.float32)
        return np.ascontiguousarray(a.reshape(-1, 128).T)

    W0 = np.asarray(inputs["pn0_W"], np.float32)
    W1 = np.asarray(inputs["pn1_W"], np.float32)
    w_pn0f = np.zeros((128, 128), np.float32)
    w_pn0f[0:64, 0:64] = W0[:, 3:].T
    w_pn0f[64:128, 64:128] = W0[:, 3:].T
    w_pn0x = np.zeros((6, 128), np.float32)
    w_pn0x[0:3, 0:64] = W0[:, 0:3].T
    w_pn0x[3:6, 64:128] = W0[:, 0:3].T

    linW = np.asarray(inputs["lin_W"], np.float32)
    featW = np.asarray(inputs["feat_W"], np.float32)
    regW1 = np.asarray(inputs["reg_W1"], np.float32)
    attW1 = np.asarray(inputs["att_W1"], np.float32)
    outW1 = np.asarray(inputs["out_W1"], np.float32)
    outW2 = np.asarray(inputs["out_W2"], np.float32)

    shared = {
        "w_pn0f": w_pn0f.astype(bf), "w_pn0x": w_pn0x.astype(bf),
        "w_pn1f": np.ascontiguousarray(W1[:, 3:].T).astype(bf),
        "w_pn1x": np.ascontiguousarray(W1[:, 0:3].T).astype(bf),
        "g_pn0": col(inputs["pn0_g"]), "bb_pn0": col(inputs["pn0_bb"]),
        "g_pn1": col(inputs["pn1_g"]), "bb_pn1": col(inputs["pn1_bb"]),
        "w_lin0": np.ascontiguousarray(linW.T[0:64]),
        "w_lin1": np.ascontiguousarray(linW.T[64:192]),
        "g_lin": colk(inputs["lin_g"]), "bb_lin": colk(inputs["lin_bb"]),
        "ti": np.ascontiguousarray(
            trace_idx.astype(np.int32).reshape(1, NA * NROT)),
        "w_feat": np.ascontiguousarray(
            featW.reshape(512, 2, 128, NA).transpose(2, 1, 3, 0)
            .reshape(128, 2 * NA * 512)).astype(bf),
        "g_feat": colk(inputs["feat_g"]), "bb_feat": colk(inputs["feat_bb"]),
        "w_reg1": np.ascontiguousarray(
            regW1.T.reshape(4, 128, 128).transpose(1, 0, 2).reshape(128, 512)),
        "g_reg": col(inputs["reg_g"]), "bb_reg": col(inputs["reg_bb"]),
        "w_att1": np.ascontiguousarray(
            attW1.T.reshape(4, 128, 128).transpose(1, 0, 2).reshape(128, 512)),
        "g_att": col(inputs["att_g"]), "bb_att": col(inputs["att_bb"]),
        "w_reg2": np.ascontiguousarray(
            np.asarray(inputs["reg_W2"], np.float32).T),
        "b_reg2": col(inputs["reg_b2"]),
        "w_att2": np.ascontiguousarray(
            np.asarray(inputs["att_W2"], np.float32).T),
        "b_att2": col(inputs["att_b2"]),
        "w_out1": np.ascontiguousarray(
            outW1.T.reshape(4, 128, 512).transpose(1, 0, 2)
            .reshape(128, 2048)),
        "g_out": colk(inputs["out_g"]), "bb_out": colk(inputs["out_bb"]),
        "w_out2": np.ascontiguousarray(
            outW2.T.reshape(4, 128, 256).transpose(1, 0, 2)
            .reshape(128, 1024)),
        "b_out2": colk(inputs["out_b2"]),
        "anch": np.ascontiguousarray(
            np.tile(anchors.reshape(NROT, 9), (BPC, 1))),
    }

    in_maps = []
    for r in range(N_CORES):
        b0 = r * BPC
        m = dict(shared)
        # f0: (b, (h,c), (p',a)) point-half packing
        f0s = feats0[b0:b0 + BPC].reshape(BPC, C0, 2, NP // 2, NA) \
            .transpose(0, 2, 1, 3, 4).reshape(BPC, 128, 6144)
        m["f0"] = np.ascontiguousarray(f0s).astype(bf)
        m["f1"] = np.ascontiguousarray(
            feats1[b0:b0 + BPC].reshape(BPC, C1, NP * NA)).astype(bf)
        x0s = xyz0[b0:b0 + BPC].reshape(BPC, 3, 2, NP // 2) \
            .transpose(0, 2, 1, 3)
        x0s = np.repeat(x0s[..., None], NA, axis=-1).reshape(BPC, 6, 6144)
        m["x0"] = np.ascontiguousarray(x0s).astype(bf)
        x1s = np.repeat(xyz1[b0:b0 + BPC][..., None], NA,
                        axis=-1).reshape(BPC, 3, NP * NA)
        m["x1"] = np.ascontiguousarray(x1s).astype(bf)
        in_maps.append(m)
    return in_maps


def _run(inputs, trace=False, debug_outs=False):
    nc = _get_nc(debug_outs=debug_outs)
    in_maps = _stage_inputs(inputs)
    res = bass_utils.run_bass_kernel_spmd(
        nc, in_maps, core_ids=list(range(N_CORES)), trace=trace)
    cat = lambda k: np.concatenate(  # noqa: E731
        [res.results[r][k] for r in range(N_CORES)])
    out = (cat("x_out").astype(np.float32), cat("x_attn").astype(np.float32),
           cat("pred_R").astype(np.float32), cat("res_T").astype(np.float32))
    return out, res


def kernel(**inputs):
    out, _ = _run(inputs, trace=False)
    return out


# revision 9
# speedup vs baseline: 1.1430x; 1.0811x over previous
# Distributed Trainium2 (8 NeuronCore) Bass kernel for nn_ArtOutBlock.
#
# Sharding: data-parallel over batch (16 batches -> 2 per core) for the heavy
# pointnet conv + max-pool phase; exact BatchNorm batch statistics via three
# small AllGathers of per-core partial bn_stats (+ pooled values), aggregated
# on-device with bn_aggr.
import os
import sys
import types

sys.path.insert(0, "/opt/trn_rl_repo")

import numpy as np
import ml_dtypes

# --- NTFF profile hook (antenv.axon_hooks is stubbed out in this image) ----
import antenv  # noqa: E402

if "antenv.axon_hooks" not in sys.modules:
    _m = types.ModuleType("antenv.axon_hooks")
    _m._hook = None
    _m.set_axon_ntff_profile_hook = lambda h: setattr(_m, "_hook", h)
    _m.get_axon_ntff_profile_hook = lambda: _m._hook
    sys.modules["antenv.axon_hooks"] = _m
    antenv.axon_hooks = _m

try:
    from trn_agent_boot.trn_boot import _ntff_profile_via_ctypes

    sys.modules["antenv.axon_hooks"].set_axon_ntff_profile_hook(
        _ntff_profile_via_ctypes("/opt/axon/libaxon_pjrt.so")
    )
except Exception:
    pass

import concourse.bass as bass  # noqa: E402,F401
import concourse.bacc as bacc  # noqa: E402
import concourse.mybir as mybir  # noqa: E402
import concourse.tile as tile  # noqa: E402
from concourse import bass_utils  # noqa: E402
from concourse.masks import make_identity  # noqa: E402

bass_utils.upload_artifacts = lambda tmpdir: f"file://{tmpdir}"

F32 = mybir.dt.float32
BF16 = mybir.dt.bfloat16
I32 = mybir.dt.int32
AX = mybir.AxisListType
ALU = mybir.AluOpType
ACTF = mybir.ActivationFunctionType

N_CORES = 8
NB, NP, NA, NROT = 16, 1024, 12, 60
C0, C1 = 64, 128
CIN, CFEAT, COUT = 256, 512, 256
BPC = NB // N_CORES  # batches per core = 2
NT = BPC * NROT      # 120
EPS = 1e-5

# bounce1 per-rank layout (f32 words)
B1_P0 = 0            # pn0 pooled, per b: 64*12      -> 2*768
B1_P1 = 1536         # pn1 pooled, per b: 128*12     -> 2*1536
B1_S0 = 4608         # pn0 bn_stats, per b: 64*6     -> 2*384
B1_S1 = 5376         # pn1 bn_stats, per b: 128*6    -> 2*768
B1_N = 6912
B2_N = 4 * 128 * 6
B3_N = 6 * 128 * 6

WEIGHT_SPECS = [
    ("w_pn0f", (128, 128), BF16), ("w_pn0x", (128, 128), BF16),
    ("w_pn1f", (128, 128), BF16), ("w_pn1x", (128, 128), BF16),
    ("g_pn0", (64, 1), F32), ("bb_pn0", (64, 1), F32),
    ("g_pn1", (128, 1), F32), ("bb_pn1", (128, 1), F32),
    ("w_lin0", (64, 256), F32), ("w_lin1", (128, 256), F32),
    ("g_lin", (128, 2), F32), ("bb_lin", (128, 2), F32),
    ("ti", (1, 720), I32),
    ("w_feat", (128, 2 * NA * CFEAT), BF16),
    ("g_feat", (128, 4), F32), ("bb_feat", (128, 4), F32),
    ("w_reg1", (128, 512), F32),
    ("g_reg", (128, 1), F32), ("bb_reg", (128, 1), F32),
    ("w_att1", (128, 512), F32),
    ("g_att", (128, 1), F32), ("bb_att", (128, 1), F32),
    ("w_reg2", (128, 7), F32), ("b_reg2", (7, 1), F32),
    ("w_att2", (128, 1), F32), ("b_att2", (1, 1), F32),
    ("w_out1", (128, 2048), F32),
    ("g_out", (128, 4), F32), ("bb_out", (128, 4), F32),
    ("w_out2", (128, 1024), F32), ("b_out2", (128, 2), F32),
    ("anch", (NT, 9), F32),
]


def _build(debug_outs=False):
    nc = bacc.Bacc("TRN2", target_bir_lowering=False, debug=False,
                   num_devices=N_CORES)
    params = {}

    def P(name, shape, dt):
        params[name] = nc.declare_dram_parameter(name, list(shape), dt,
                                                 isOutput=False)

    P("f0", (BPC, 128, 6144), BF16)   # (b, (h,c), (p',a))
    P("f1", (BPC, 128, NP * NA), BF16)
    P("x0", (BPC, 6, 6144), BF16)   # (b, (h,c), (p',a)) pre-broadcast
    P("x1", (BPC, 3, NP * NA), BF16)
    for name, shape, dt in WEIGHT_SPECS:
        P(name, shape, dt)

    outs = {
        "x_out": nc.declare_dram_parameter("x_out", [BPC, COUT], F32, True),
        "x_attn": nc.declare_dram_parameter("x_attn", [BPC, NROT], F32, True),
        "pred_R": nc.declare_dram_parameter("pred_R", [BPC, NROT, 3, 3], F32,
                                            True),
        "res_T": nc.declare_dram_parameter("res_T", [BPC, NROT, 3], F32, True),
    }
    if debug_outs:
        outs["dbg_pool"] = nc.declare_dram_parameter(
            "dbg_pool", [BPC, 192, NA], F32, True)
        outs["dbg_xf"] = nc.declare_dram_parameter(
            "dbg_xf", [128, NT], F32, True)
        outs["dbg_lin"] = nc.declare_dram_parameter(
            "dbg_lin", [BPC, NA, 256], F32, True)

    with tile.TileContext(nc) as tc:
        _graph(nc, tc, params, outs)
    nc.finalize()
    return nc


def _graph(nc, tc, prm, outs):
    from contextlib import ExitStack

    RG = [list(range(N_CORES))]
    ctx = ExitStack()
    with ctx:
        consts = ctx.enter_context(tc.tile_pool(name="consts", bufs=1))
        wpool = ctx.enter_context(tc.tile_pool(name="wpool", bufs=1))
        small = ctx.enter_context(tc.tile_pool(name="small", bufs=1))
        dram = ctx.enter_context(tc.tile_pool(name="dram", bufs=1,
                                              space="DRAM"))

        ident = consts.tile([128, 128], F32, tag="ident")
        make_identity(nc, ident)

        # register float constants used as activation biases
        for cv in (EPS, float(np.pi / 2.0), float(-np.pi / 10.0)):
            ct = consts.tile([128, 1], F32, tag=f"cst{cv}")
            nc.gpsimd.memset(ct, cv)
            nc.const_aps.aps[(F32, cv)] = ct

        PH1 = {"w_pn0f", "w_pn0x", "w_pn1f", "w_pn1x",
               "g_pn0", "bb_pn0", "g_pn1", "bb_pn1"}
        W = {}
        for name, shape, dt in WEIGHT_SPECS:
            if name in PH1:
                tl = wpool.tile(list(shape), dt, tag=name, name=f"w_{name}")
                nc.sync.dma_start(tl, prm[name].ap())
                W[name] = tl


        # ================= phase 1: convs + max pool =================
        bigin = ctx.enter_context(tc.tile_pool(name="bigin", bufs=2))
        # static zero-padded xyz rhs tiles (K=128 keeps the PE fast path)
        xz0 = wpool.tile([128, 6144], BF16, tag="xz0")
        nc.vector.memset(xz0, 0.0)
        xz1 = wpool.tile([128, NP * NA], BF16, tag="xz1")
        nc.vector.memset(xz1, 0.0)
        pools = {}

        def conv_level(kind, b):
            if kind == "pn0":
                src = prm["f0"].ap()[b]      # (128, 6144)
                ncols, nslots = 6144, 4
                wf, wx, xt = W["w_pn0f"], W["w_pn0x"], xz0
                nc.sync.dma_start(xz0[0:6, :], prm["x0"].ap()[b])
            else:
                src = prm["f1"].ap()[b]      # (128, 12288)
                ncols, nslots = NP * NA, 8
                wf, wx, xt = W["w_pn1f"], W["w_pn1x"], xz1
                nc.sync.dma_start(xz1[0:3, :], prm["x1"].ap()[b])

            inp = bigin.tile([128, ncols], BF16, tag="conv_in")
            nc.sync.dma_start(inp, src)
            slots = small.tile([128, nslots * NA], F32,
                               tag=f"slots_{kind}_{b}")
            with tc.tile_pool(name=f"cps_{kind}_{b}", bufs=2,
                              space="PSUM") as cpp:
                for s in range(nslots):  # each slot: 128 points
                    ps = cpp.tile([128, 2048], F32, tag="cpsum")
                    for m in range(4):   # 4 matmuls x 32 points
                        pbase = s * 128 + m * 32
                        nc.tensor.matmul(ps[:, m * 512:m * 512 + 384],
                                         wf, inp[:, pbase * NA:(pbase + 32) * NA],
                                         start=True, stop=False)
                        nc.tensor.matmul(
                            ps[:, m * 512:m * 512 + 384], wx,
                            xt[:, pbase * NA:(pbase + 32) * NA],
                            start=False, stop=True)
                    red = ps.rearrange("q (m c) -> q m c", m=4)[:, :, 0:384] \
                        .rearrange("q m (p a) -> q m p a", a=NA) \
                        .transpose([0, 3, 1, 2])  # (128, 12, 4, 32)
                    nc.vector.tensor_reduce(slots[:, s * NA:(s + 1) * NA],
                                            red, axis=AX.XY, op=ALU.max)
            pooled = small.tile([128, NA], F32, tag=f"pool_{kind}_{b}")
            nc.vector.tensor_reduce(
                pooled,
                slots.rearrange("q (s a) -> q s a", a=NA).transpose([0, 2, 1]),
                axis=AX.X, op=ALU.max)
            return pooled

        for b in range(BPC):
            pools[("pn0", b)] = conv_level("pn0", b)
        for b in range(BPC):
            pools[("pn1", b)] = conv_level("pn1", b)

        # phase-2 weights: issued after conv loads, on the scalar HWDGE queue
        for name, shape, dt in WEIGHT_SPECS:
            if name not in PH1:
                tl = wpool.tile(list(shape), dt, tag=name, name=f"w_{name}")
                nc.scalar.dma_start(tl, prm[name].ap())
                W[name] = tl

        # pn0: combine point-halves (partitions (h,o)) via transpose
        p0 = {}
        with tc.tile_pool(name="tps", bufs=2, space="PSUM") as tps:
            for b in range(BPC):
                pt = tps.tile([NA, 128], F32, tag="tp")
                nc.tensor.transpose(pt, pools[("pn0", b)], ident)
                pts = small.tile([NA, 128], F32, tag=f"pts{b}")
                nc.scalar.copy(pts, pt)
                hm = small.tile([NA, 64], F32, tag=f"hmax{b}")
                nc.vector.tensor_tensor(hm, pts[:, 0:64], pts[:, 64:128],
                                        op=ALU.max)
                bt = tps.tile([64, NA], F32, tag="tpb")
                nc.tensor.transpose(bt, hm, ident[0:NA, 0:NA])
                p0b = small.tile([64, NA], F32, tag=f"p0_{b}")
                nc.scalar.copy(p0b, bt)
                p0[b] = p0b

        stats1 = {}
        for b in range(BPC):
            s0 = small.tile([64, 6], F32, tag=f"s0_{b}")
            nc.vector.bn_stats(s0, p0[b])
            stats1[("pn0", b)] = s0
            s1 = small.tile([128, 6], F32, tag=f"s1_{b}")
            nc.vector.bn_stats(s1, pools[("pn1", b)])
            stats1[("pn1", b)] = s1

        # ---- bounce1 + AllGather 1 ----
        b1in = dram.tile([B1_N], F32, tag="b1in")
        b1out = dram.tile([N_CORES, B1_N], F32, tag="b1out")
        for b in range(BPC):
            nc.sync.dma_start(
                b1in[B1_P0 + b * 768:B1_P0 + (b + 1) * 768]
                .rearrange("(c a) -> c a", a=NA), p0[b])
            nc.sync.dma_start(
                b1in[B1_P1 + b * 1536:B1_P1 + (b + 1) * 1536]
                .rearrange("(c a) -> c a", a=NA), pools[("pn1", b)])
            nc.sync.dma_start(
                b1in[B1_S0 + b * 384:B1_S0 + (b + 1) * 384]
                .rearrange("(c k) -> c k", k=6), stats1[("pn0", b)])
            nc.sync.dma_start(
                b1in[B1_S1 + b * 768:B1_S1 + (b + 1) * 768]
                .rearrange("(c k) -> c k", k=6), stats1[("pn1", b)])
        nc.gpsimd.collective_compute(
            "AllGather", ALU.bypass, replica_groups=RG,
            ins=[b1in.opt()], outs=[b1out.opt()])
        g1 = b1out  # (8, B1_N)

        def agg_from(srcs, parts, tag):
            # srcs: list of 3-dim (parts, 8, 6) dram views
            st = small.tile([parts, len(srcs), 8, 6], F32, tag=f"aggin_{tag}",
                            name=f"aggin_{tag}")
            for i, sv in enumerate(srcs):
                nc.sync.dma_start(st[:, i], sv)
            ag = small.tile([parts, 2], F32, tag=f"agg_{tag}",
                            name=f"agg_{tag}")
            nc.vector.bn_aggr(ag, st)
            return ag

        def stat_view(off, nchan, b):
            return g1[:, off + b * nchan * 6:off + (b + 1) * nchan * 6] \
                .rearrange("r (c k) -> r c k", k=6).transpose([1, 0, 2])

        agg_pn0 = agg_from([stat_view(B1_S0, 64, b) for b in range(2)],
                           64, "pn0")
        agg_pn1 = agg_from([stat_view(B1_S1, 128, b) for b in range(2)],
                           128, "pn1")

        def mk_scale_shift(agg, g_t, bb_t, parts, tag):
            sd = small.tile([parts, 1], F32, tag=f"sd_{tag}")
            nc.scalar.activation(sd, agg[:, 1:2], ACTF.Sqrt, bias=EPS)
            rs = small.tile([parts, 1], F32, tag=f"rs_{tag}")
            nc.vector.reciprocal(rs, sd)
            s = small.tile([parts, 1], F32, tag=f"s_{tag}")
            nc.vector.tensor_tensor(s, rs, g_t, op=ALU.mult)
            ms = small.tile([parts, 1], F32, tag=f"ms_{tag}")
            nc.vector.tensor_tensor(ms, agg[:, 0:1], s, op=ALU.mult)
            sh = small.tile([parts, 1], F32, tag=f"sh_{tag}")
            nc.vector.tensor_tensor(sh, bb_t, ms, op=ALU.subtract)
            return s, sh

        s_pn0, t_pn0 = mk_scale_shift(agg_pn0, W["g_pn0"], W["bb_pn0"],
                                      64, "pn0")
        s_pn1, t_pn1 = mk_scale_shift(agg_pn1, W["g_pn1"], W["bb_pn1"],
                                      128, "pn1")

        # ---- all-batch pooled -> lin stats ----
        tileA = small.tile([64, 2, 8, NA], F32, tag="tileA")
        tileB = small.tile([128, 2, 8, NA], F32, tag="tileB")
        for b in range(2):
            nc.sync.dma_start(
                tileA[:, b],
                g1[:, B1_P0 + b * 768:B1_P0 + (b + 1) * 768]
                .rearrange("r (c a) -> r c a", a=NA).transpose([1, 0, 2]))
            nc.sync.dma_start(
                tileB[:, b],
                g1[:, B1_P1 + b * 1536:B1_P1 + (b + 1) * 1536]
                .rearrange("r (c a) -> r c a", a=NA).transpose([1, 0, 2]))
        nc.scalar.activation(tileA, tileA, ACTF.Relu, scale=s_pn0, bias=t_pn0)
        nc.scalar.activation(tileB, tileB, ACTF.Relu, scale=s_pn1, bias=t_pn1)

        ps2 = ctx.enter_context(tc.tile_pool(name="ps2", bufs=4, space="PSUM"))
        psf = ctx.enter_context(tc.tile_pool(name="psf", bufs=4, space="PSUM"))

        lin_s, lin_t = [], []
        for oc in range(2):
            pl = ps2.tile([128, 192], F32, tag="mm")
            nc.tensor.matmul(pl, W["w_lin0"][:, oc * 128:(oc + 1) * 128],
                             tileA.rearrange("c b r a -> c (b r a)"),
                             start=True, stop=False)
            nc.tensor.matmul(pl, W["w_lin1"][:, oc * 128:(oc + 1) * 128],
                             tileB.rearrange("c b r a -> c (b r a)"),
                             start=False, stop=True)
            st = small.tile([128, 6], F32, tag=f"linst{oc}")
            nc.vector.bn_stats(st, pl)
            ag = small.tile([128, 2], F32, tag=f"linag{oc}")
            nc.vector.bn_aggr(ag, st)
            s2, t2 = mk_scale_shift(ag, W["g_lin"][:, oc:oc + 1],
                                    W["bb_lin"][:, oc:oc + 1],
                                    128, f"lin{oc}")
            lin_s.append(s2)
            lin_t.append(t2)

        # ---- own-batch pooled BN -> lin -> transpose ----
        p0bn, p1bn = {}, {}
        for b in range(BPC):
            a0 = small.tile([64, NA], F32, tag=f"p0bn{b}")
            nc.scalar.activation(a0, p0[b], ACTF.Relu, scale=s_pn0, bias=t_pn0)
            p0bn[b] = a0
            a1 = small.tile([128, NA], F32, tag=f"p1bn{b}")
            nc.scalar.activation(a1, pools[("pn1", b)], ACTF.Relu,
                                 scale=s_pn1, bias=t_pn1)
            p1bn[b] = a1

        linT = {b: small.tile([NA, 256], BF16, tag=f"linT{b}",
                               name=f"linT{b}") for b in range(BPC)}
        lin_bn_dbg = {}
        for b in range(BPC):
            for oc in range(2):
                pl = ps2.tile([128, NA], F32, tag="mm")
                nc.tensor.matmul(pl, W["w_lin0"][:, oc * 128:(oc + 1) * 128],
                                 p0bn[b], start=True, stop=False)
                nc.tensor.matmul(pl, W["w_lin1"][:, oc * 128:(oc + 1) * 128],
                                 p1bn[b], start=False, stop=True)
                lb = small.tile([128, NA], F32, tag=f"linbn{b}{oc}")
                nc.scalar.activation(lb, pl, ACTF.Identity,
                                     scale=lin_s[oc], bias=lin_t[oc])
                lin_bn_dbg[(b, oc)] = lb
                pt = ps2.tile([NA, 128], F32, tag="mm")
                nc.tensor.transpose(pt, lb, ident)
                nc.scalar.copy(linT[b][:, oc * 128:(oc + 1) * 128], pt)

        # ---- one-hot from trace_idx ----
        ti_f1 = small.tile([1, 720], F32, tag="ti_f1")
        nc.vector.tensor_copy(ti_f1, W["ti"])
        ti_f = small.tile([NA, 720], F32, tag="ti_f")
        nc.gpsimd.partition_broadcast(ti_f, ti_f1)
        io_i = small.tile([NA, 720], I32, tag="io_i")
        nc.gpsimd.iota(io_i, pattern=[[0, 720]], base=0, channel_multiplier=1)
        io_f = small.tile([NA, 720], F32, tag="io_f")
        nc.vector.tensor_copy(io_f, io_i)
        oh = small.tile([NA, 720], BF16, tag="oh")
        nc.vector.tensor_tensor(oh, ti_f, io_f, op=ALU.is_equal)

        # ---- gather matmuls -> X2 ----
        X2 = [small.tile([128, BPC * 720], BF16, tag=f"X2_{cc}",
                         name=f"X2_{cc}") for cc in range(2)]
        for b in range(BPC):
            for cc in range(2):
                for j in range(2):
                    pg = ps2.tile([128, 360], F32, tag="mm")
                    nc.tensor.matmul(pg, linT[b][:, cc * 128:(cc + 1) * 128],
                                     oh[:, j * 360:(j + 1) * 360],
                                     start=True, stop=True)
                    nc.scalar.copy(
                        X2[cc][:, b * 720 + j * 360:b * 720 + (j + 1) * 360],
                        pg)

        # ---- feat matmuls ----
        wf_v = W["w_feat"].rearrange("q (cc a o) -> q cc a o", cc=2, a=NA)
        feat_ps = []
        for oc in range(4):
            pf = psf.tile([128, NT], F32, tag="featmm")
            first = True
            for cc in range(2):
                for a in range(NA):
                    rhs = X2[cc].rearrange("q (b a r) -> q b a r",
                                           b=BPC, a=NA)[:, :, a, :]
                    nc.tensor.matmul(pf,
                                     wf_v[:, cc, a, oc * 128:(oc + 1) * 128],
                                     rhs, start=first,
                                     stop=(cc == 1 and a == NA - 1))
                    first = False
            feat_ps.append(pf)

        # ---- feat stats -> AllGather 2 ----
        b2in = dram.tile([B2_N], F32, tag="b2in")
        b2out = dram.tile([N_CORES, B2_N], F32, tag="b2out")
        for oc in range(4):
            fs = small.tile([128, 6], F32, tag=f"fstat{oc}")
            nc.vector.bn_stats(fs, feat_ps[oc])
            nc.sync.dma_start(
                b2in[oc * 768:(oc + 1) * 768].rearrange("(c k) -> c k", k=6),
                fs)
        nc.gpsimd.collective_compute(
            "AllGather", ALU.bypass, replica_groups=RG,
            ins=[b2in.opt()], outs=[b2out.opt()])

        xf, xm = [], []
        for oc in range(4):
            ag = agg_from(
                [b2out[:, oc * 768:(oc + 1) * 768]
                 .rearrange("r (c k) -> r c k", k=6).transpose([1, 0, 2])],
                128, f"feat{oc}")
            s3, t3 = mk_scale_shift(ag, W["g_feat"][:, oc:oc + 1],
                                    W["bb_feat"][:, oc:oc + 1],
                                    128, f"feat{oc}")
            x = small.tile([128, NT], F32, tag=f"xf{oc}")
            nc.scalar.activation(x, feat_ps[oc], ACTF.Relu, scale=s3, bias=t3)
            xf.append(x)
            m = small.tile([128, BPC], F32, tag=f"xm{oc}")
            nc.vector.tensor_reduce(m, x.rearrange("q (b r) -> q b r", b=BPC),
                                    axis=AX.X, op=ALU.max)
            xm.append(m)

        # ---- reg1 / att1 / out1 ----
        wr1 = W["w_reg1"].rearrange("q (k o) -> q k o", k=4)
        wa1 = W["w_att1"].rearrange("q (k o) -> q k o", k=4)
        wo1 = W["w_out1"].rearrange("q (k o) -> q k o", k=4)
        p_reg = ps2.tile([128, NT], F32, tag="mm")
        p_att = ps2.tile([128, NT], F32, tag="mm")
        for k in range(4):
            nc.tensor.matmul(p_reg, wr1[:, k, :], xf[k],
                             start=(k == 0), stop=(k == 3))
        for k in range(4):
            nc.tensor.matmul(p_att, wa1[:, k, :], xf[k],
                             start=(k == 0), stop=(k == 3))
        p_out1 = []
        for oc in range(4):
            po = psf.tile([128, BPC], F32, tag="featmm")
            for k in range(4):
                nc.tensor.matmul(po, wo1[:, k, oc * 128:(oc + 1) * 128],
                                 xm[k], start=(k == 0), stop=(k == 3))
            p_out1.append(po)

        # ---- stats -> AllGather 3 ----
        b3in = dram.tile([B3_N], F32, tag="b3in")
        b3out = dram.tile([N_CORES, B3_N], F32, tag="b3out")

        def put_stats(ps_t, off, tag):
            st = small.tile([128, 6], F32, tag=f"st3_{tag}")
            nc.vector.bn_stats(st, ps_t)
            nc.sync.dma_start(
                b3in[off:off + 768].rearrange("(c k) -> c k", k=6), st)

        put_stats(p_reg, 0, "reg")
        put_stats(p_att, 768, "att")
        for oc in range(4):
            put_stats(p_out1[oc], 1536 + oc * 768, f"o1{oc}")
        nc.gpsimd.collective_compute(
            "AllGather", ALU.bypass, replica_groups=RG,
            ins=[b3in.opt()], outs=[b3out.opt()])

        def agg3(off, g_t, bb_t, tag):
            ag = agg_from(
                [b3out[:, off:off + 768]
                 .rearrange("r (c k) -> r c k", k=6).transpose([1, 0, 2])],
                128, f"a3_{tag}")
            return mk_scale_shift(ag, g_t, bb_t, 128, f"h_{tag}")

        s4, t4 = agg3(0, W["g_reg"], W["bb_reg"], "reg")
        s5, t5 = agg3(768, W["g_att"], W["bb_att"], "att")
        h_reg = small.tile([128, NT], F32, tag="h_reg")
        nc.scalar.activation(h_reg, p_reg, ACTF.Relu, scale=s4, bias=t4)
        a_att = small.tile([128, NT], F32, tag="a_att")
        nc.scalar.activation(a_att, p_att, ACTF.Relu, scale=s5, bias=t5)
        o_o = []
        for oc in range(4):
            s6, t6 = agg3(1536 + oc * 768, W["g_out"][:, oc:oc + 1],
                          W["bb_out"][:, oc:oc + 1], f"o1{oc}")
            oo = small.tile([128, BPC], F32, tag=f"oo{oc}")
            nc.scalar.activation(oo, p_out1[oc], ACTF.Relu, scale=s6, bias=t6)
            o_o.append(oo)

        # ---- reg2 / att2 / out2 ----
        p_r2 = ps2.tile([7, NT], F32, tag="mm")
        nc.tensor.matmul(p_r2, W["w_reg2"], h_reg, start=True, stop=True)
        resid7 = small.tile([7, NT], F32, tag="resid7")
        nc.scalar.activation(resid7, p_r2, ACTF.Identity, bias=W["b_reg2"])
        p_rt = ps2.tile([NT, 7], F32, tag="mm")
        nc.tensor.transpose(p_rt, resid7, ident[0:7, 0:7])
        rT = small.tile([NT, 7], F32, tag="rT")
        nc.scalar.copy(rT, p_rt)

        p_a2 = ps2.tile([1, NT], F32, tag="mm")
        nc.tensor.matmul(p_a2, W["w_att2"], a_att, start=True, stop=True)
        logits = small.tile([1, NT], F32, tag="logits")
        nc.scalar.activation(logits, p_a2, ACTF.Identity, bias=W["b_att2"])
        lmax = small.tile([1, BPC], F32, tag="lmax")
        nc.vector.tensor_reduce(lmax,
                                logits.rearrange("q (b r) -> q b r", b=BPC),
                                axis=AX.X, op=ALU.max)
        lsh = small.tile([1, NT], F32, tag="lsh")
        nc.vector.tensor_tensor(
            lsh.rearrange("q (b r) -> q b r", b=BPC),
            logits.rearrange("q (b r) -> q b r", b=BPC),
            lmax.unsqueeze(2).broadcast_to([1, BPC, NROT]), op=ALU.subtract)
        lexp = small.tile([1, NT], F32, tag="lexp")
        nc.scalar.activation(lexp, lsh, ACTF.Exp)
        lsum = small.tile([1, BPC], F32, tag="lsum")
        nc.vector.tensor_reduce(lsum,
                                lexp.rearrange("q (b r) -> q b r", b=BPC),
                                axis=AX.X, op=ALU.add)
        lrec = small.tile([1, BPC], F32, tag="lrec")
        nc.vector.reciprocal(lrec, lsum)
        attn = small.tile([1, NT], F32, tag="attn")
        nc.vector.tensor_tensor(
            attn.rearrange("q (b r) -> q b r", b=BPC),
            lexp.rearrange("q (b r) -> q b r", b=BPC),
            lrec.unsqueeze(2).broadcast_to([1, BPC, NROT]), op=ALU.mult)

        wo2 = W["w_out2"].rearrange("q (k o) -> q k o", k=4)
        xout_sb = []
        for oc in range(2):
            po = ps2.tile([128, BPC], F32, tag="mm")
            for k in range(4):
                nc.tensor.matmul(po, wo2[:, k, oc * 128:(oc + 1) * 128],
                                 o_o[k], start=(k == 0), stop=(k == 3))
            xs = small.tile([128, BPC], F32, tag=f"xout{oc}")
            nc.scalar.activation(xs, po, ACTF.Identity,
                                 bias=W["b_out2"][:, oc:oc + 1])
            xout_sb.append(xs)

        # ---- SO(3) exp map + pred_R ----
        d3 = rT[:, 0:3]
        sq = small.tile([NT, 3], F32, tag="sq")
        nc.vector.tensor_tensor(sq, d3, d3, op=ALU.mult)
        nrm2 = small.tile([NT, 1], F32, tag="nrm2")
        nc.vector.tensor_reduce(nrm2, sq, axis=AX.X, op=ALU.add)
        nrm = small.tile([NT, 1], F32, tag="nrm")
        nc.scalar.activation(nrm, nrm2, ACTF.Sqrt)
        ninv = small.tile([NT, 1], F32, tag="ninv")
        nc.vector.reciprocal(ninv, nrm)
        D = small.tile([NT, 3], F32, tag="D")
        nc.vector.tensor_scalar_mul(D, d3, ninv)
        sg = small.tile([NT, 1], F32, tag="sg")
        nc.scalar.activation(sg, rT[:, 3:4], ACTF.Sigmoid)
        Nv = small.tile([NT, 1], F32, tag="Nv")
        nc.scalar.activation(Nv, sg, ACTF.Identity,
                             scale=np.pi / 5.0, bias=-np.pi / 10.0)
        th = small.tile([NT, 1], F32, tag="th")
        nc.scalar.activation(th, Nv, ACTF.Abs)
        sgn = small.tile([NT, 1], F32, tag="sgn")
        nc.scalar.sign(sgn, Nv)
        kv = small.tile([NT, 3], F32, tag="kv")
        nc.vector.tensor_scalar_mul(kv, D, sgn)
        sin_t = small.tile([NT, 1], F32, tag="sin_t")
        nc.scalar.activation(sin_t, th, ACTF.Sin)
        cos_t = small.tile([NT, 1], F32, tag="cos_t")
        nc.scalar.activation(cos_t, th, ACTF.Sin, bias=np.pi / 2.0)
        cm1 = small.tile([NT, 1], F32, tag="cm1")
        nc.vector.tensor_scalar(cm1, cos_t, scalar1=-1.0, scalar2=1.0,
                                op0=ALU.mult, op1=ALU.add)
        R9 = small.tile([NT, 9], F32, tag="R9")
        nc.vector.tensor_tensor(
            R9.rearrange("q (i j) -> q i j", i=3),
            kv.unsqueeze(2).broadcast_to([NT, 3, 3]),
            kv.unsqueeze(1).broadcast_to([NT, 3, 3]), op=ALU.mult)
        nc.vector.tensor_scalar_mul(R9, R9, cm1)
        sk = small.tile([NT, 3], F32, tag="sk")
        nc.vector.tensor_scalar_mul(sk, kv, sin_t)
        for (slot, comp, sign) in ((1, 2, -1), (2, 1, 1), (3, 2, 1),
                                   (5, 0, -1), (6, 1, -1), (7, 0, 1)):
            op = ALU.add if sign > 0 else ALU.subtract
            nc.vector.tensor_tensor(R9[:, slot:slot + 1],
                                    R9[:, slot:slot + 1],
                                    sk[:, comp:comp + 1], op=op)
        for d in range(3):
            nc.vector.tensor_tensor(R9[:, 4 * d:4 * d + 1],
                                    R9[:, 4 * d:4 * d + 1], cos_t,
                                    op=ALU.add)
        predR = small.tile([NT, 9], F32, tag="predR")
        tmp9 = small.tile([NT, 9], F32, tag="tmp9")
        A3 = W["anch"].rearrange("q (i j) -> q i j", i=3)
        R3 = R9.rearrange("q (i j) -> q i j", i=3)
        for j in range(3):
            a_ij = A3[:, :, j].unsqueeze(2).broadcast_to([NT, 3, 3])
            r_jk = R3[:, j, :].unsqueeze(1).broadcast_to([NT, 3, 3])
            if j == 0:
                nc.vector.tensor_tensor(
                    predR.rearrange("q (i k) -> q i k", i=3),
                    a_ij, r_jk, op=ALU.mult)
            else:
                nc.vector.tensor_tensor(
                    tmp9.rearrange("q (i k) -> q i k", i=3),
                    a_ij, r_jk, op=ALU.mult)
                nc.vector.tensor_tensor(predR, predR, tmp9, op=ALU.add)

        # ---- outputs ----
        for oc in range(2):
            nc.sync.dma_start(
                outs["x_out"].ap()[:, oc * 128:(oc + 1) * 128]
                .transpose([1, 0]), xout_sb[oc])
        nc.sync.dma_start(
            outs["x_attn"].ap().rearrange("b r -> (b r)").unsqueeze(0), attn)
        nc.sync.dma_start(
            outs["pred_R"].ap().rearrange("b r i j -> (b r) (i j)"), predR)
        nc.sync.dma_start(
            outs["res_T"].ap().rearrange("b r k -> (b r) k"), rT[:, 4:7])

        if "dbg_pool" in outs:
            dp = outs["dbg_pool"].ap()
            for b in range(BPC):
                nc.sync.dma_start(dp[b, 0:64, :], p0[b])
                nc.sync.dma_start(dp[b, 64:192, :], pools[("pn1", b)])
            nc.sync.dma_start(outs["dbg_xf"].ap(), xf[0])
            dl = outs["dbg_lin"].ap()
            for b in range(BPC):
                nc.sync.dma_start(dl[b], linT[b])


_NC_CACHE = {}


def _get_nc(debug_outs=False):
    key = bool(debug_outs)
    if key not in _NC_CACHE:
        _NC_CACHE[key] = _build(debug_outs=key)
    return _NC_CACHE[key]


def _stage_inputs(inputs):
    bf = ml_dtypes.bfloat16
    xyz0 = np.asarray(inputs["xyz0"], np.float32)
    feats0 = np.asarray(inputs["feats0"], np.float32)
    xyz1 = np.asarray(inputs["xyz1"], np.float32)
    feats1 = np.asarray(inputs["feats1"], np.float32)
    trace_idx = np.asarray(inputs["trace_idx"])
    anchors = np.asarray(inputs["anchors"], np.float32)

    def col(v):
        return np.ascontiguousarray(np.asarray(v, np.float32).reshape(-1, 1))

    def colk(v):
        a = np.asarray(v, np.float32)
        return np.ascontiguousarray(a.reshape(-1, 128).T)

    W0 = np.asarray(inputs["pn0_W"], np.float32)
    W1 = np.asarray(inputs["pn1_W"], np.float32)
    w_pn0f = np.zeros((128, 128), np.float32)
    w_pn0f[0:64, 0:64] = W0[:, 3:].T
    w_pn0f[64:128, 64:128] = W0[:, 3:].T
    w_pn0x = np.zeros((128, 128), np.float32)
    w_pn0x[0:3, 0:64] = W0[:, 0:3].T
    w_pn0x[3:6, 64:128] = W0[:, 0:3].T
    w_pn1x = np.zeros((128, 128), np.float32)
    w_pn1x[0:3, :] = W1[:, 0:3].T

    linW = np.asarray(inputs["lin_W"], np.float32)
    featW = np.asarray(inputs["feat_W"], np.float32)
    regW1 = np.asarray(inputs["reg_W1"], np.float32)
    attW1 = np.asarray(inputs["att_W1"], np.float32)
    outW1 = np.asarray(inputs["out_W1"], np.float32)
    outW2 = np.asarray(inputs["out_W2"], np.float32)

    shared = {
        "w_pn0f": w_pn0f.astype(bf), "w_pn0x": w_pn0x.astype(bf),
        "w_pn1f": np.ascontiguousarray(W1[:, 3:].T).astype(bf),
        "w_pn1x": w_pn1x.astype(bf),
        "g_pn0": col(inputs["pn0_g"]), "bb_pn0": col(inputs["pn0_bb"]),
        "g_pn1": col(inputs["pn1_g"]), "bb_pn1": col(inputs["pn1_bb"]),
        "w_lin0": np.ascontiguousarray(linW.T[0:64]),
        "w_lin1": np.ascontiguousarray(linW.T[64:192]),
        "g_lin": colk(inputs["lin_g"]), "bb_lin": colk(inputs["lin_bb"]),
        "ti": np.ascontiguousarray(
            trace_idx.astype(np.int32).reshape(1, NA * NROT)),
        "w_feat": np.ascontiguousarray(
            featW.reshape(512, 2, 128, NA).transpose(2, 1, 3, 0)
            .reshape(128, 2 * NA * 512)).astype(bf),
        "g_feat": colk(inputs["feat_g"]), "bb_feat": colk(inputs["feat_bb"]),
        "w_reg1": np.ascontiguousarray(
            regW1.T.reshape(4, 128, 128).transpose(1, 0, 2).reshape(128, 512)),
        "g_reg": col(inputs["reg_g"]), "bb_reg": col(inputs["reg_bb"]),
        "w_att1": np.ascontiguousarray(
            attW1.T.reshape(4, 128, 128).transpose(1, 0, 2).reshape(128, 512)),
        "g_att": col(inputs["att_g"]), "bb_att": col(inputs["att_bb"]),
        "w_reg2": np.ascontiguousarray(
            np.asarray(inputs["reg_W2"], np.float32).T),
        "b_reg2": col(inputs["reg_b2"]),
        "w_att2": np.ascontiguousarray(
            np.asarray(inputs["att_W2"], np.float32).T),
        "b_att2": col(inputs["att_b2"]),
        "w_out1": np.ascontiguousarray(
            outW1.T.reshape(4, 128, 512).transpose(1, 0, 2)
            .reshape(128, 2048)),
        "g_out": colk(inputs["out_g"]), "bb_out": colk(inputs["out_bb"]),
        "w_out2": np.ascontiguousarray(
            outW2.T.reshape(4, 128, 256).transpose(1, 0, 2)
            .reshape(128, 1024)),
        "b_out2": colk(inputs["out_b2"]),
        "anch": np.ascontiguousarray(
            np.tile(anchors.reshape(NROT, 9), (BPC, 1))),
    }

    in_maps = []
    for r in range(N_CORES):
        b0 = r * BPC
        m = dict(shared)
        # f0: (b, (h,c), (p',a)) point-half packing
        f0s = feats0[b0:b0 + BPC].reshape(BPC, C0, 2, NP // 2, NA) \
            .transpose(0, 2, 1, 3, 4).reshape(BPC, 128, 6144)
        m["f0"] = np.ascontiguousarray(f0s).astype(bf)
        m["f1"] = np.ascontiguousarray(
            feats1[b0:b0 + BPC].reshape(BPC, C1, NP * NA)).astype(bf)
        x0s = xyz0[b0:b0 + BPC].reshape(BPC, 3, 2, NP // 2) \
            .transpose(0, 2, 1, 3)
        x0s = np.repeat(x0s[..., None], NA, axis=-1).reshape(BPC, 6, 6144)
        m["x0"] = np.ascontiguousarray(x0s).astype(bf)
        x1s = np.repeat(xyz1[b0:b0 + BPC][..., None], NA,
                        axis=-1).reshape(BPC, 3, NP * NA)
        m["x1"] = np.ascontiguousarray(x1s).astype(bf)
        in_maps.append(m)
    return in_maps


def _run(inputs, trace=False, debug_outs=False):
    nc = _get_nc(debug_outs=debug_outs)
    in_maps = _stage_inputs(inputs)
    res = bass_utils.run_bass_kernel_spmd(
        nc, in_maps, core_ids=list(range(N_CORES)), trace=trace)
    cat = lambda k: np.concatenate(  # noqa: E731
        [res.results[r][k] for r in range(N_CORES)])
    out = (cat("x_out").astype(np.float32), cat("x_attn").astype(np.float32),
           cat("pred_R").astype(np.float32), cat("res_T").astype(np.float32))
    return out, res


def kernel(**inputs):
    out, _ = _run(inputs, trace=False)
    return out


# revision 12
# speedup vs baseline: 1.1700x; 1.0237x over previous
# Distributed Trainium2 (8 NeuronCore) Bass kernel for nn_ArtOutBlock.
#
# Sharding: data-parallel over batch (16 batches -> 2 per core) for the heavy
# pointnet conv + max-pool phase; exact BatchNorm batch statistics via three
# small AllGathers of per-core partial bn_stats (+ pooled values), aggregated
# on-device with bn_aggr.
import os
import sys
import types

sys.path.insert(0, "/opt/trn_rl_repo")

import numpy as np
import ml_dtypes

# --- NTFF profile hook (antenv.axon_hooks is stubbed out in this image) ----
import antenv  # noqa: E402

if "antenv.axon_hooks" not in sys.modules:
    _m = types.ModuleType("antenv.axon_hooks")
    _m._hook = None
    _m.set_axon_ntff_profile_hook = lambda h: setattr(_m, "_hook", h)
    _m.get_axon_ntff_profile_hook = lambda: _m._hook
    sys.modules["antenv.axon_hooks"] = _m
    antenv.axon_hooks = _m

try:
    from trn_agent_boot.trn_boot import _ntff_profile_via_ctypes

    sys.modules["antenv.axon_hooks"].set_axon_ntff_profile_hook(
        _ntff_profile_via_ctypes("/opt/axon/libaxon_pjrt.so")
    )
except Exception:
    pass

import concourse.bass as bass  # noqa: E402,F401
import concourse.bacc as bacc  # noqa: E402
import concourse.mybir as mybir  # noqa: E402
import concourse.tile as tile  # noqa: E402
from concourse import bass_utils  # noqa: E402
from concourse.masks import make_identity  # noqa: E402

bass_utils.upload_artifacts = lambda tmpdir: f"file://{tmpdir}"

F32 = mybir.dt.float32
BF16 = mybir.dt.bfloat16
I32 = mybir.dt.int32
AX = mybir.AxisListType
ALU = mybir.AluOpType
ACTF = mybir.ActivationFunctionType

N_CORES = 8
NB, NP, NA, NROT = 16, 1024, 12, 60
C0, C1 = 64, 128
CIN, CFEAT, COUT = 256, 512, 256
BPC = NB // N_CORES  # batches per core = 2
NT = BPC * NROT      # 120
EPS = 1e-5

# bounce1a per-rank layout (f32 words): pn0 (both b) + pn1 b0
A_P0 = 0             # pn0 pooled, per b: 64*12      -> 2*768
A_S0 = 1536          # pn0 bn_stats, per b: 64*6     -> 2*384
A_P1 = 2304          # pn1 b0 pooled: 128*12
A_S1 = 3840          # pn1 b0 bn_stats: 128*6
B1A_N = 4608
# bounce1b per-rank layout: pn1 b1
B_P1 = 0             # pn1 b1 pooled: 128*12
B_S1 = 1536          # pn1 b1 bn_stats: 128*6
B1B_N = 2304
B2_N = 4 * 128 * 6
B3_N = 6 * 128 * 6

WEIGHT_SPECS = [
    ("w_pn0f", (128, 128), BF16), ("w_pn0x", (128, 128), BF16),
    ("w_pn1f", (128, 128), BF16), ("w_pn1x", (128, 128), BF16),
    ("g_pn0", (64, 1), F32), ("bb_pn0", (64, 1), F32),
    ("g_pn1", (128, 1), F32), ("bb_pn1", (128, 1), F32),
    ("w_lin0", (64, 256), F32), ("w_lin1", (128, 256), F32),
    ("g_lin", (128, 2), F32), ("bb_lin", (128, 2), F32),
    ("ti", (1, 720), I32),
    ("w_feat", (128, 2 * NA * CFEAT), BF16),
    ("g_feat", (128, 4), F32), ("bb_feat", (128, 4), F32),
    ("w_reg1", (128, 512), F32),
    ("g_reg", (128, 1), F32), ("bb_reg", (128, 1), F32),
    ("w_att1", (128, 512), F32),
    ("g_att", (128, 1), F32), ("bb_att", (128, 1), F32),
    ("w_reg2", (128, 7), F32), ("b_reg2", (7, 1), F32),
    ("w_att2", (128, 1), F32), ("b_att2", (1, 1), F32),
    ("w_out1", (128, 2048), F32),
    ("g_out", (128, 4), F32), ("bb_out", (128, 4), F32),
    ("w_out2", (128, 1024), F32), ("b_out2", (128, 2), F32),
    ("anch", (NT, 9), F32),
]


def _build(debug_outs=False):
    nc = bacc.Bacc("TRN2", target_bir_lowering=False, debug=False,
                   num_devices=N_CORES)
    params = {}

    def P(name, shape, dt):
        params[name] = nc.declare_dram_parameter(name, list(shape), dt,
                                                 isOutput=False)

    P("f0", (BPC, 128, 6144), BF16)   # (b, (h,c), (p',a))
    P("f1", (BPC, 128, NP * NA), BF16)
    P("x0", (BPC, 6, 6144), BF16)   # (b, (h,c), (p',a)) pre-broadcast
    P("x1", (BPC, 3, NP * NA), BF16)
    for name, shape, dt in WEIGHT_SPECS:
        P(name, shape, dt)

    outs = {
        "x_out": nc.declare_dram_parameter("x_out", [BPC, COUT], F32, True),
        "x_attn": nc.declare_dram_parameter("x_attn", [BPC, NROT], F32, True),
        "pred_R": nc.declare_dram_parameter("pred_R", [BPC, NROT, 3, 3], F32,
                                            True),
        "res_T": nc.declare_dram_parameter("res_T", [BPC, NROT, 3], F32, True),
    }
    if debug_outs:
        outs["dbg_pool"] = nc.declare_dram_parameter(
            "dbg_pool", [BPC, 192, NA], F32, True)
        outs["dbg_xf"] = nc.declare_dram_parameter(
            "dbg_xf", [128, NT], F32, True)
        outs["dbg_lin"] = nc.declare_dram_parameter(
            "dbg_lin", [BPC, NA, 256], F32, True)

    with tile.TileContext(nc) as tc:
        _graph(nc, tc, params, outs)
    nc.finalize()
    return nc


def _graph(nc, tc, prm, outs):
    from contextlib import ExitStack

    RG = [list(range(N_CORES))]
    ctx = ExitStack()
    with ctx:
        consts = ctx.enter_context(tc.tile_pool(name="consts", bufs=1))
        wpool = ctx.enter_context(tc.tile_pool(name="wpool", bufs=1))
        small = ctx.enter_context(tc.tile_pool(name="small", bufs=1))
        dram = ctx.enter_context(tc.tile_pool(name="dram", bufs=1,
                                              space="DRAM"))

        ident = consts.tile([128, 128], F32, tag="ident")
        make_identity(nc, ident)

        # register float constants used as activation biases
        for cv in (EPS, float(np.pi / 2.0), float(-np.pi / 10.0)):
            ct = consts.tile([128, 1], F32, tag=f"cst{cv}")
            nc.gpsimd.memset(ct, cv)
            nc.const_aps.aps[(F32, cv)] = ct

        PH1 = {"w_pn0f", "w_pn0x", "w_pn1f", "w_pn1x",
               "g_pn0", "bb_pn0", "g_pn1", "bb_pn1"}
        W = {}
        for name, shape, dt in WEIGHT_SPECS:
            if name in PH1:
                tl = wpool.tile(list(shape), dt, tag=name, name=f"w_{name}")
                nc.sync.dma_start(tl, prm[name].ap())
                W[name] = tl


        # ================= phase 1: convs + max pool =================
        bigin = ctx.enter_context(tc.tile_pool(name="bigin", bufs=2))
        # static zero-padded xyz rhs tiles (K=128 keeps the PE fast path)
        xz0 = wpool.tile([128, 6144], BF16, tag="xz0")
        nc.vector.memset(xz0, 0.0)
        xz1 = wpool.tile([128, NP * NA], BF16, tag="xz1")
        nc.vector.memset(xz1, 0.0)
        pools = {}

        def conv_level(kind, b):
            if kind == "pn0":
                src = prm["f0"].ap()[b]      # (128, 6144)
                ncols, nslots = 6144, 4
                wf, wx, xt = W["w_pn0f"], W["w_pn0x"], xz0
                nc.sync.dma_start(xz0[0:6, :], prm["x0"].ap()[b])
            else:
                src = prm["f1"].ap()[b]      # (128, 12288)
                ncols, nslots = NP * NA, 8
                wf, wx, xt = W["w_pn1f"], W["w_pn1x"], xz1
                nc.sync.dma_start(xz1[0:3, :], prm["x1"].ap()[b])

            inp = bigin.tile([128, ncols], BF16, tag="conv_in")
            nc.sync.dma_start(inp, src)
            slots = small.tile([128, nslots * NA], F32,
                               tag=f"slots_{kind}_{b}")
            if True:
                for s in range(nslots):  # each slot: 128 points
                    ps = cps.tile([128, 2048], F32, tag="cpsum")
                    for m in range(4):   # 4 matmuls x 32 points
                        pbase = s * 128 + m * 32
                        nc.tensor.matmul(ps[:, m * 512:m * 512 + 384],
                                         wf, inp[:, pbase * NA:(pbase + 32) * NA],
                                         start=True, stop=False)
                        nc.tensor.matmul(
                            ps[:, m * 512:m * 512 + 384], wx,
                            xt[:, pbase * NA:(pbase + 32) * NA],
                            start=False, stop=True)
                    red = ps.rearrange("q (m c) -> q m c", m=4)[:, :, 0:384] \
                        .rearrange("q m (p a) -> q m p a", a=NA) \
                        .transpose([0, 3, 1, 2])  # (128, 12, 4, 32)
                    nc.vector.tensor_reduce(slots[:, s * NA:(s + 1) * NA],
                                            red, axis=AX.XY, op=ALU.max)
            pooled = small.tile([128, NA], F32, tag=f"pool_{kind}_{b}")
            nc.vector.tensor_reduce(
                pooled,
                slots.rearrange("q (s a) -> q s a", a=NA).transpose([0, 2, 1]),
                axis=AX.X, op=ALU.max)
            return pooled

        p0 = {}
        stats1 = {}
        ph1 = ExitStack()
        cps = ph1.enter_context(tc.tile_pool(name="cps", bufs=2,
                                             space="PSUM"))

        def pn0_combine(b):
            pt = cps.tile([NA, 128], F32, tag="cpsum", name=f"tp{b}")
            nc.tensor.transpose(pt, pools[("pn0", b)], ident)
            pts = small.tile([NA, 128], F32, tag=f"pts{b}", name=f"pts{b}")
            nc.scalar.copy(pts, pt)
            hm = small.tile([NA, 64], F32, tag=f"hmax{b}", name=f"hm{b}")
            nc.vector.tensor_tensor(hm, pts[:, 0:64], pts[:, 64:128],
                                    op=ALU.max)
            bt = cps.tile([64, NA], F32, tag="cpsum", name=f"bt{b}")
            nc.tensor.transpose(bt, hm, ident[0:NA, 0:NA])
            p0b = small.tile([64, NA], F32, tag=f"p0_{b}", name=f"p0_{b}")
            nc.scalar.copy(p0b, bt)
            p0[b] = p0b
            s0 = small.tile([64, 6], F32, tag=f"s0_{b}", name=f"s0_{b}")
            nc.vector.bn_stats(s0, p0[b])
            stats1[("pn0", b)] = s0

        b1ain = dram.tile([B1A_N], F32, tag="b1ain")
        b1aout = dram.tile([N_CORES, B1A_N], F32, tag="b1aout")
        b1bin = dram.tile([B1B_N], F32, tag="b1bin")
        b1bout = dram.tile([N_CORES, B1B_N], F32, tag="b1bout")

        for b in range(BPC):
            pools[("pn0", b)] = conv_level("pn0", b)
            pn0_combine(b)
        pools[("pn1", 0)] = conv_level("pn1", 0)
        s1 = small.tile([128, 6], F32, tag="s1_0")
        nc.vector.bn_stats(s1, pools[("pn1", 0)])
        stats1[("pn1", 0)] = s1
        # AG1a: pn0 (both) + pn1 b0 -- overlaps pn1 b1 convs
        for b in range(BPC):
            nc.sync.dma_start(
                b1ain[A_P0 + b * 768:A_P0 + (b + 1) * 768]
                .rearrange("(c a) -> c a", a=NA), p0[b])
            nc.sync.dma_start(
                b1ain[A_S0 + b * 384:A_S0 + (b + 1) * 384]
                .rearrange("(c k) -> c k", k=6), stats1[("pn0", b)])
        nc.sync.dma_start(
            b1ain[A_P1:A_P1 + 1536].rearrange("(c a) -> c a", a=NA),
            pools[("pn1", 0)])
        nc.sync.dma_start(
            b1ain[A_S1:A_S1 + 768].rearrange("(c k) -> c k", k=6),
            stats1[("pn1", 0)])
        nc.gpsimd.collective_compute(
            "AllGather", ALU.bypass, replica_groups=RG,
            ins=[b1ain.opt()], outs=[b1aout.opt()])

        pools[("pn1", 1)] = conv_level("pn1", 1)
        s1b = small.tile([128, 6], F32, tag="s1_1")
        nc.vector.bn_stats(s1b, pools[("pn1", 1)])
        stats1[("pn1", 1)] = s1b
        nc.sync.dma_start(
            b1bin[B_P1:B_P1 + 1536].rearrange("(c a) -> c a", a=NA),
            pools[("pn1", 1)])
        nc.sync.dma_start(
            b1bin[B_S1:B_S1 + 768].rearrange("(c k) -> c k", k=6),
            stats1[("pn1", 1)])
        nc.gpsimd.collective_compute(
            "AllGather", ALU.bypass, replica_groups=RG,
            ins=[b1bin.opt()], outs=[b1bout.opt()])

        # phase-2 weights: issued after conv loads, on the scalar HWDGE queue
        for name, shape, dt in WEIGHT_SPECS:
            if name not in PH1:
                tl = wpool.tile(list(shape), dt, tag=name, name=f"w_{name}")
                nc.scalar.dma_start(tl, prm[name].ap())
                W[name] = tl
        ph1.close()

        def agg_from(srcs, parts, tag):
            # srcs: list of 3-dim (parts, 8, 6) dram views
            st = small.tile([parts, len(srcs), 8, 6], F32, tag=f"aggin_{tag}",
                            name=f"aggin_{tag}")
            for i, sv in enumerate(srcs):
                nc.sync.dma_start(st[:, i], sv)
            ag = small.tile([parts, 2], F32, tag=f"agg_{tag}",
                            name=f"agg_{tag}")
            nc.vector.bn_aggr(ag, st)
            return ag

        def stat_view(buf, off, nchan):
            return buf[:, off:off + nchan * 6] \
                .rearrange("r (c k) -> r c k", k=6).transpose([1, 0, 2])

        agg_pn0 = agg_from([stat_view(b1aout, A_S0 + b * 384, 64)
                            for b in range(2)], 64, "pn0")
        agg_pn1 = agg_from([stat_view(b1aout, A_S1, 128),
                            stat_view(b1bout, B_S1, 128)], 128, "pn1")

        def mk_scale_shift(agg, g_t, bb_t, parts, tag):
            sd = small.tile([parts, 1], F32, tag=f"sd_{tag}")
            nc.scalar.activation(sd, agg[:, 1:2], ACTF.Sqrt, bias=EPS)
            rs = small.tile([parts, 1], F32, tag=f"rs_{tag}")
            nc.vector.reciprocal(rs, sd)
            s = small.tile([parts, 1], F32, tag=f"s_{tag}")
            nc.vector.tensor_tensor(s, rs, g_t, op=ALU.mult)
            ms = small.tile([parts, 1], F32, tag=f"ms_{tag}")
            nc.vector.tensor_tensor(ms, agg[:, 0:1], s, op=ALU.mult)
            sh = small.tile([parts, 1], F32, tag=f"sh_{tag}")
            nc.vector.tensor_tensor(sh, bb_t, ms, op=ALU.subtract)
            return s, sh

        s_pn0, t_pn0 = mk_scale_shift(agg_pn0, W["g_pn0"], W["bb_pn0"],
                                      64, "pn0")
        s_pn1, t_pn1 = mk_scale_shift(agg_pn1, W["g_pn1"], W["bb_pn1"],
                                      128, "pn1")

        # ---- all-batch pooled -> lin stats ----
        tileA = small.tile([64, 2, 8, NA], F32, tag="tileA")
        tileB = small.tile([128, 2, 8, NA], F32, tag="tileB")
        for b in range(2):
            nc.sync.dma_start(
                tileA[:, b],
                b1aout[:, A_P0 + b * 768:A_P0 + (b + 1) * 768]
                .rearrange("r (c a) -> r c a", a=NA).transpose([1, 0, 2]))
        nc.sync.dma_start(
            tileB[:, 0],
            b1aout[:, A_P1:A_P1 + 1536]
            .rearrange("r (c a) -> r c a", a=NA).transpose([1, 0, 2]))
        nc.sync.dma_start(
            tileB[:, 1],
            b1bout[:, B_P1:B_P1 + 1536]
            .rearrange("r (c a) -> r c a", a=NA).transpose([1, 0, 2]))
        nc.scalar.activation(tileA, tileA, ACTF.Relu, scale=s_pn0, bias=t_pn0)
        nc.scalar.activation(tileB, tileB, ACTF.Relu, scale=s_pn1, bias=t_pn1)

        ps2 = ctx.enter_context(tc.tile_pool(name="ps2", bufs=4, space="PSUM"))
        psf = ctx.enter_context(tc.tile_pool(name="psf", bufs=4, space="PSUM"))

        lin_s, lin_t = [], []
        for oc in range(2):
            pl = ps2.tile([128, 192], F32, tag="mm")
            nc.tensor.matmul(pl, W["w_lin0"][:, oc * 128:(oc + 1) * 128],
                             tileA.rearrange("c b r a -> c (b r a)"),
                             start=True, stop=False)
            nc.tensor.matmul(pl, W["w_lin1"][:, oc * 128:(oc + 1) * 128],
                             tileB.rearrange("c b r a -> c (b r a)"),
                             start=False, stop=True)
            st = small.tile([128, 6], F32, tag=f"linst{oc}")
            nc.vector.bn_stats(st, pl)
            ag = small.tile([128, 2], F32, tag=f"linag{oc}")
            nc.vector.bn_aggr(ag, st)
            s2, t2 = mk_scale_shift(ag, W["g_lin"][:, oc:oc + 1],
                                    W["bb_lin"][:, oc:oc + 1],
                                    128, f"lin{oc}")
            lin_s.append(s2)
            lin_t.append(t2)

        # ---- own-batch pooled BN -> lin -> transpose ----
        p0bn, p1bn = {}, {}
        for b in range(BPC):
            a0 = small.tile([64, NA], F32, tag=f"p0bn{b}")
            nc.scalar.activation(a0, p0[b], ACTF.Relu, scale=s_pn0, bias=t_pn0)
            p0bn[b] = a0
            a1 = small.tile([128, NA], F32, tag=f"p1bn{b}")
            nc.scalar.activation(a1, pools[("pn1", b)], ACTF.Relu,
                                 scale=s_pn1, bias=t_pn1)
            p1bn[b] = a1

        linT = {b: small.tile([NA, 256], BF16, tag=f"linT{b}",
                               name=f"linT{b}") for b in range(BPC)}
        lin_bn_dbg = {}
        for b in range(BPC):
            for oc in range(2):
                pl = ps2.tile([128, NA], F32, tag="mm")
                nc.tensor.matmul(pl, W["w_lin0"][:, oc * 128:(oc + 1) * 128],
                                 p0bn[b], start=True, stop=False)
                nc.tensor.matmul(pl, W["w_lin1"][:, oc * 128:(oc + 1) * 128],
                                 p1bn[b], start=False, stop=True)
                lb = small.tile([128, NA], F32, tag=f"linbn{b}{oc}")
                nc.scalar.activation(lb, pl, ACTF.Identity,
                                     scale=lin_s[oc], bias=lin_t[oc])
                lin_bn_dbg[(b, oc)] = lb
                pt = ps2.tile([NA, 128], F32, tag="mm")
                nc.tensor.transpose(pt, lb, ident)
                nc.scalar.copy(linT[b][:, oc * 128:(oc + 1) * 128], pt)

        # ---- one-hot from trace_idx ----
        ti_f1 = small.tile([1, 720], F32, tag="ti_f1")
        nc.vector.tensor_copy(ti_f1, W["ti"])
        ti_f = small.tile([NA, 720], F32, tag="ti_f")
        nc.gpsimd.partition_broadcast(ti_f, ti_f1)
        io_i = small.tile([NA, 720], I32, tag="io_i")
        nc.gpsimd.iota(io_i, pattern=[[0, 720]], base=0, channel_multiplier=1)
        io_f = small.tile([NA, 720], F32, tag="io_f")
        nc.vector.tensor_copy(io_f, io_i)
        oh = small.tile([NA, 720], BF16, tag="oh")
        nc.vector.tensor_tensor(oh, ti_f, io_f, op=ALU.is_equal)

        # ---- gather matmuls -> X2 ----
        X2 = [small.tile([128, BPC * 720], BF16, tag=f"X2_{cc}",
                         name=f"X2_{cc}") for cc in range(2)]
        for b in range(BPC):
            for cc in range(2):
                for j in range(2):
                    pg = ps2.tile([128, 360], F32, tag="mm")
                    nc.tensor.matmul(pg, linT[b][:, cc * 128:(cc + 1) * 128],
                                     oh[:, j * 360:(j + 1) * 360],
                                     start=True, stop=True)
                    nc.scalar.copy(
                        X2[cc][:, b * 720 + j * 360:b * 720 + (j + 1) * 360],
                        pg)

        # ---- feat matmuls ----
        wf_v = W["w_feat"].rearrange("q (cc a o) -> q cc a o", cc=2, a=NA)
        feat_ps = []
        for oc in range(4):
            pf = psf.tile([128, NT], F32, tag="featmm")
            first = True
            for cc in range(2):
                for a in range(NA):
                    rhs = X2[cc].rearrange("q (b a r) -> q b a r",
                                           b=BPC, a=NA)[:, :, a, :]
                    nc.tensor.matmul(pf,
                                     wf_v[:, cc, a, oc * 128:(oc + 1) * 128],
                                     rhs, start=first,
                                     stop=(cc == 1 and a == NA - 1))
                    first = False
            feat_ps.append(pf)

        # ---- feat stats -> AllGather 2 ----
        b2in = dram.tile([B2_N], F32, tag="b2in")
        b2out = dram.tile([N_CORES, B2_N], F32, tag="b2out")
        for oc in range(4):
            fs = small.tile([128, 6], F32, tag=f"fstat{oc}")
            nc.vector.bn_stats(fs, feat_ps[oc])
            nc.sync.dma_start(
                b2in[oc * 768:(oc + 1) * 768].rearrange("(c k) -> c k", k=6),
                fs)
        nc.gpsimd.collective_compute(
            "AllGather", ALU.bypass, replica_groups=RG,
            ins=[b2in.opt()], outs=[b2out.opt()])

        xf, xm = [], []
        for oc in range(4):
            ag = agg_from(
                [b2out[:, oc * 768:(oc + 1) * 768]
                 .rearrange("r (c k) -> r c k", k=6).transpose([1, 0, 2])],
                128, f"feat{oc}")
            s3, t3 = mk_scale_shift(ag, W["g_feat"][:, oc:oc + 1],
                                    W["bb_feat"][:, oc:oc + 1],
                                    128, f"feat{oc}")
            x = small.tile([128, NT], F32, tag=f"xf{oc}")
            nc.scalar.activation(x, feat_ps[oc], ACTF.Relu, scale=s3, bias=t3)
            xf.append(x)
            m = small.tile([128, BPC], F32, tag=f"xm{oc}")
            nc.vector.tensor_reduce(m, x.rearrange("q (b r) -> q b r", b=BPC),
                                    axis=AX.X, op=ALU.max)
            xm.append(m)

        # ---- reg1 / att1 / out1 ----
        wr1 = W["w_reg1"].rearrange("q (k o) -> q k o", k=4)
        wa1 = W["w_att1"].rearrange("q (k o) -> q k o", k=4)
        wo1 = W["w_out1"].rearrange("q (k o) -> q k o", k=4)
        p_reg = ps2.tile([128, NT], F32, tag="mm")
        p_att = ps2.tile([128, NT], F32, tag="mm")
        for k in range(4):
            nc.tensor.matmul(p_reg, wr1[:, k, :], xf[k],
                             start=(k == 0), stop=(k == 3))
        for k in range(4):
            nc.tensor.matmul(p_att, wa1[:, k, :], xf[k],
                             start=(k == 0), stop=(k == 3))
        p_out1 = []
        for oc in range(4):
            po = psf.tile([128, BPC], F32, tag="featmm")
            for k in range(4):
                nc.tensor.matmul(po, wo1[:, k, oc * 128:(oc + 1) * 128],
                                 xm[k], start=(k == 0), stop=(k == 3))
            p_out1.append(po)

        # ---- stats -> AllGather 3 ----
        b3in = dram.tile([B3_N], F32, tag="b3in")
        b3out = dram.tile([N_CORES, B3_N], F32, tag="b3out")

        def put_stats(ps_t, off, tag):
            st = small.tile([128, 6], F32, tag=f"st3_{tag}")
            nc.vector.bn_stats(st, ps_t)
            nc.sync.dma_start(
                b3in[off:off + 768].rearrange("(c k) -> c k", k=6), st)

        put_stats(p_reg, 0, "reg")
        put_stats(p_att, 768, "att")
        for oc in range(4):
            put_stats(p_out1[oc], 1536 + oc * 768, f"o1{oc}")
        nc.gpsimd.collective_compute(
            "AllGather", ALU.bypass, replica_groups=RG,
            ins=[b3in.opt()], outs=[b3out.opt()])

        def agg3(off, g_t, bb_t, tag):
            ag = agg_from(
                [b3out[:, off:off + 768]
                 .rearrange("r (c k) -> r c k", k=6).transpose([1, 0, 2])],
                128, f"a3_{tag}")
            return mk_scale_shift(ag, g_t, bb_t, 128, f"h_{tag}")

        s4, t4 = agg3(0, W["g_reg"], W["bb_reg"], "reg")
        s5, t5 = agg3(768, W["g_att"], W["bb_att"], "att")
        h_reg = small.tile([128, NT], F32, tag="h_reg")
        nc.scalar.activation(h_reg, p_reg, ACTF.Relu, scale=s4, bias=t4)
        a_att = small.tile([128, NT], F32, tag="a_att")
        nc.scalar.activation(a_att, p_att, ACTF.Relu, scale=s5, bias=t5)
        o_o = []
        for oc in range(4):
            s6, t6 = agg3(1536 + oc * 768, W["g_out"][:, oc:oc + 1],
                          W["bb_out"][:, oc:oc + 1], f"o1{oc}")
            oo = small.tile([128, BPC], F32, tag=f"oo{oc}")
            nc.scalar.activation(oo, p_out1[oc], ACTF.Relu, scale=s6, bias=t6)
            o_o.append(oo)

        # ---- reg2 / att2 / out2 ----
        p_r2 = ps2.tile([7, NT], F32, tag="mm")
        nc.tensor.matmul(p_r2, W["w_reg2"], h_reg, start=True, stop=True)
        resid7 = small.tile([7, NT], F32, tag="resid7")
        nc.scalar.activation(resid7, p_r2, ACTF.Identity, bias=W["b_reg2"])
        p_rt = ps2.tile([NT, 7], F32, tag="mm")
        nc.tensor.transpose(p_rt, resid7, ident[0:7, 0:7])
        rT = small.tile([NT, 7], F32, tag="rT")
        nc.scalar.copy(rT, p_rt)

        p_a2 = ps2.tile([1, NT], F32, tag="mm")
        nc.tensor.matmul(p_a2, W["w_att2"], a_att, start=True, stop=True)
        logits = small.tile([1, NT], F32, tag="logits")
        nc.scalar.activation(logits, p_a2, ACTF.Identity, bias=W["b_att2"])
        lmax = small.tile([1, BPC], F32, tag="lmax")
        nc.vector.tensor_reduce(lmax,
                                logits.rearrange("q (b r) -> q b r", b=BPC),
                                axis=AX.X, op=ALU.max)
        lsh = small.tile([1, NT], F32, tag="lsh")
        nc.vector.tensor_tensor(
            lsh.rearrange("q (b r) -> q b r", b=BPC),
            logits.rearrange("q (b r) -> q b r", b=BPC),
            lmax.unsqueeze(2).broadcast_to([1, BPC, NROT]), op=ALU.subtract)
        lexp = small.tile([1, NT], F32, tag="lexp")
        nc.scalar.activation(lexp, lsh, ACTF.Exp)
        lsum = small.tile([1, BPC], F32, tag="lsum")
        nc.vector.tensor_reduce(lsum,
                                lexp.rearrange("q (b r) -> q b r", b=BPC),
                                axis=AX.X, op=ALU.add)
        lrec = small.tile([1, BPC], F32, tag="lrec")
        nc.vector.reciprocal(lrec, lsum)
        attn = small.tile([1, NT], F32, tag="attn")
        nc.vector.tensor_tensor(
            attn.rearrange("q (b r) -> q b r", b=BPC),
            lexp.rearrange("q (b r) -> q b r", b=BPC),
            lrec.unsqueeze(2).broadcast_to([1, BPC, NROT]), op=ALU.mult)

        wo2 = W["w_out2"].rearrange("q (k o) -> q k o", k=4)
        xout_sb = []
        for oc in range(2):
            po = ps2.tile([128, BPC], F32, tag="mm")
            for k in range(4):
                nc.tensor.matmul(po, wo2[:, k, oc * 128:(oc + 1) * 128],
                                 o_o[k], start=(k == 0), stop=(k == 3))
            xs = small.tile([128, BPC], F32, tag=f"xout{oc}")
            nc.scalar.activation(xs, po, ACTF.Identity,
                                 bias=W["b_out2"][:, oc:oc + 1])
            xout_sb.append(xs)

        # ---- SO(3) exp map + pred_R ----
        d3 = rT[:, 0:3]
        sq = small.tile([NT, 3], F32, tag="sq")
        nc.vector.tensor_tensor(sq, d3, d3, op=ALU.mult)
        nrm2 = small.tile([NT, 1], F32, tag="nrm2")
        nc.vector.tensor_reduce(nrm2, sq, axis=AX.X, op=ALU.add)
        nrm = small.tile([NT, 1], F32, tag="nrm")
        nc.scalar.activation(nrm, nrm2, ACTF.Sqrt)
        ninv = small.tile([NT, 1], F32, tag="ninv")
        nc.vector.reciprocal(ninv, nrm)
        D = small.tile([NT, 3], F32, tag="D")
        nc.vector.tensor_scalar_mul(D, d3, ninv)
        sg = small.tile([NT, 1], F32, tag="sg")
        nc.scalar.activation(sg, rT[:, 3:4], ACTF.Sigmoid)
        Nv = small.tile([NT, 1], F32, tag="Nv")
        nc.scalar.activation(Nv, sg, ACTF.Identity,
                             scale=np.pi / 5.0, bias=-np.pi / 10.0)
        th = small.tile([NT, 1], F32, tag="th")
        nc.scalar.activation(th, Nv, ACTF.Abs)
        sgn = small.tile([NT, 1], F32, tag="sgn")
        nc.scalar.sign(sgn, Nv)
        kv = small.tile([NT, 3], F32, tag="kv")
        nc.vector.tensor_scalar_mul(kv, D, sgn)
        sin_t = small.tile([NT, 1], F32, tag="sin_t")
        nc.scalar.activation(sin_t, th, ACTF.Sin)
        cos_t = small.tile([NT, 1], F32, tag="cos_t")
        nc.scalar.activation(cos_t, th, ACTF.Sin, bias=np.pi / 2.0)
        cm1 = small.tile([NT, 1], F32, tag="cm1")
        nc.vector.tensor_scalar(cm1, cos_t, scalar1=-1.0, scalar2=1.0,
                                op0=ALU.mult, op1=ALU.add)
        R9 = small.tile([NT, 9], F32, tag="R9")
        nc.vector.tensor_tensor(
            R9.rearrange("q (i j) -> q i j", i=3),
            kv.unsqueeze(2).broadcast_to([NT, 3, 3]),
            kv.unsqueeze(1).broadcast_to([NT, 3, 3]), op=ALU.mult)
        nc.vector.tensor_scalar_mul(R9, R9, cm1)
        sk = small.tile([NT, 3], F32, tag="sk")
        nc.vector.tensor_scalar_mul(sk, kv, sin_t)
        for (slot, comp, sign) in ((1, 2, -1), (2, 1, 1), (3, 2, 1),
                                   (5, 0, -1), (6, 1, -1), (7, 0, 1)):
            op = ALU.add if sign > 0 else ALU.subtract
            nc.vector.tensor_tensor(R9[:, slot:slot + 1],
                                    R9[:, slot:slot + 1],
                                    sk[:, comp:comp + 1], op=op)
        for d in range(3):
            nc.vector.tensor_tensor(R9[:, 4 * d:4 * d + 1],
                                    R9[:, 4 * d:4 * d + 1], cos_t,
                                    op=ALU.add)
        predR = small.tile([NT, 9], F32, tag="predR")
        tmp9 = small.tile([NT, 9], F32, tag="tmp9")
        A3 = W["anch"].rearrange("q (i j) -> q i j", i=3)
        R3 = R9.rearrange("q (i j) -> q i j", i=3)
        for j in range(3):
            a_ij = A3[:, :, j].unsqueeze(2).broadcast_to([NT, 3, 3])
            r_jk = R3[:, j, :].unsqueeze(1).broadcast_to([NT, 3, 3])
            if j == 0:
                nc.vector.tensor_tensor(
                    predR.rearrange("q (i k) -> q i k", i=3),
                    a_ij, r_jk, op=ALU.mult)
            else:
                nc.vector.tensor_tensor(
                    tmp9.rearrange("q (i k) -> q i k", i=3),
                    a_ij, r_jk, op=ALU.mult)
                nc.vector.tensor_tensor(predR, predR, tmp9, op=ALU.add)

        # ---- outputs ----
        for oc in range(2):
            nc.sync.dma_start(
                outs["x_out"].ap()[:, oc * 128:(oc + 1) * 128]
                .transpose([1, 0]), xout_sb[oc])
        nc.sync.dma_start(
            outs["x_attn"].ap().rearrange("b r -> (b r)").unsqueeze(0), attn)
        nc.sync.dma_start(
            outs["pred_R"].ap().rearrange("b r i j -> (b r) (i j)"), predR)
        nc.sync.dma_start(
            outs["res_T"].ap().rearrange("b r k -> (b r) k"), rT[:, 4:7])

        if "dbg_pool" in outs:
            dp = outs["dbg_pool"].ap()
            for b in range(BPC):
                nc.sync.dma_start(dp[b, 0:64, :], p0[b])
                nc.sync.dma_start(dp[b, 64:192, :], pools[("pn1", b)])
            nc.sync.dma_start(outs["dbg_xf"].ap(), xf[0])
            dl = outs["dbg_lin"].ap()
            for b in range(BPC):
                nc.sync.dma_start(dl[b], linT[b])


_NC_CACHE = {}


def _get_nc(debug_outs=False):
    key = bool(debug_outs)
    if key not in _NC_CACHE:
        _NC_CACHE[key] = _build(debug_outs=key)
    return _NC_CACHE[key]


def _stage_inputs(inputs):
    bf = ml_dtypes.bfloat16
    xyz0 = np.asarray(inputs["xyz0"], np.float32)
    feats0 = np.asarray(inputs["feats0"], np.float32)
    xyz1 = np.asarray(inputs["xyz1"], np.float32)
    feats1 = np.asarray(inputs["feats1"], np.float32)
    trace_idx = np.asarray(inputs["trace_idx"])
    anchors = np.asarray(inputs["anchors"], np.float32)

    def col(v):
        return np.ascontiguousarray(np.asarray(v, np.float32).reshape(-1, 1))

    def colk(v):
        a = np.asarray(v, np.float32)
        return np.ascontiguousarray(a.reshape(-1, 128).T)

    W0 = np.asarray(inputs["pn0_W"], np.float32)
    W1 = np.asarray(inputs["pn1_W"], np.float32)
    w_pn0f = np.zeros((128, 128), np.float32)
    w_pn0f[0:64, 0:64] = W0[:, 3:].T
    w_pn0f[64:128, 64:128] = W0[:, 3:].T
    w_pn0x = np.zeros((128, 128), np.float32)
    w_pn0x[0:3, 0:64] = W0[:, 0:3].T
    w_pn0x[3:6, 64:128] = W0[:, 0:3].T
    w_pn1x = np.zeros((128, 128), np.float32)
    w_pn1x[0:3, :] = W1[:, 0:3].T

    linW = np.asarray(inputs["lin_W"], np.float32)
    featW = np.asarray(inputs["feat_W"], np.float32)
    regW1 = np.asarray(inputs["reg_W1"], np.float32)
    attW1 = np.asarray(inputs["att_W1"], np.float32)
    outW1 = np.asarray(inputs["out_W1"], np.float32)
    outW2 = np.asarray(inputs["out_W2"], np.float32)

    shared = {
        "w_pn0f": w_pn0f.astype(bf), "w_pn0x": w_pn0x.astype(bf),
        "w_pn1f": np.ascontiguousarray(W1[:, 3:].T).astype(bf),
        "w_pn1x": w_pn1x.astype(bf),
        "g_pn0": col(inputs["pn0_g"]), "bb_pn0": col(inputs["pn0_bb"]),
        "g_pn1": col(inputs["pn1_g"]), "bb_pn1": col(inputs["pn1_bb"]),
        "w_lin0": np.ascontiguousarray(linW.T[0:64]),
        "w_lin1": np.ascontiguousarray(linW.T[64:192]),
        "g_lin": colk(inputs["lin_g"]), "bb_lin": colk(inputs["lin_bb"]),
        "ti": np.ascontiguousarray(
            trace_idx.astype(np.int32).reshape(1, NA * NROT)),
        "w_feat": np.ascontiguousarray(
            featW.reshape(512, 2, 128, NA).transpose(2, 1, 3, 0)
            .reshape(128, 2 * NA * 512)).astype(bf),
        "g_feat": colk(inputs["feat_g"]), "bb_feat": colk(inputs["feat_bb"]),
        "w_reg1": np.ascontiguousarray(
            regW1.T.reshape(4, 128, 128).transpose(1, 0, 2).reshape(128, 512)),
        "g_reg": col(inputs["reg_g"]), "bb_reg": col(inputs["reg_bb"]),
        "w_att1": np.ascontiguousarray(
            attW1.T.reshape(4, 128, 128).transpose(1, 0, 2).reshape(128, 512)),
        "g_att": col(inputs["att_g"]), "bb_att": col(inputs["att_bb"]),
        "w_reg2": np.ascontiguousarray(
            np.asarray(inputs["reg_W2"], np.float32).T),
        "b_reg2": col(inputs["reg_b2"]),
        "w_att2": np.ascontiguousarray(
            np.asarray(inputs["att_W2"], np.float32).T),
        "b_att2": col(inputs["att_b2"]),
        "w_out1": np.ascontiguousarray(
            outW1.T.reshape(4, 128, 512).transpose(1, 0, 2)
            .reshape(128, 2048)),
        "g_out": colk(inputs["out_g"]), "bb_out": colk(inputs["out_bb"]),
        "w_out2": np.ascontiguousarray(
            outW2.T.reshape(4, 128, 256).transpose(1, 0, 2)
            .reshape(128, 1024)),
        "b_out2": colk(inputs["out_b2"]),
        "anch": np.ascontiguousarray(
            np.tile(anchors.reshape(NROT, 9), (BPC, 1))),
    }

    in_maps = []
    for r in range(N_CORES):
        b0 = r * BPC
        m = dict(shared)
        # f0: (b, (h,c), (p',a)) point-half packing
        f0s = feats0[b0:b0 + BPC].reshape(BPC, C0, 2, NP // 2, NA) \
            .transpose(0, 2, 1, 3, 4).reshape(BPC, 128, 6144)
        m["f0"] = np.ascontiguousarray(f0s).astype(bf)
        m["f1"] = np.ascontiguousarray(
            feats1[b0:b0 + BPC].reshape(BPC, C1, NP * NA)).astype(bf)
        x0s = xyz0[b0:b0 + BPC].reshape(BPC, 3, 2, NP // 2) \
            .transpose(0, 2, 1, 3)
        x0s = np.repeat(x0s[..., None], NA, axis=-1).reshape(BPC, 6, 6144)
        m["x0"] = np.ascontiguousarray(x0s).astype(bf)
        x1s = np.repeat(xyz1[b0:b0 + BPC][..., None], NA,
                        axis=-1).reshape(BPC, 3, NP * NA)
        m["x1"] = np.ascontiguousarray(x1s).astype(bf)
        in_maps.append(m)
    return in_maps


def _run(inputs, trace=False, debug_outs=False):
    nc = _get_nc(debug_outs=debug_outs)
    in_maps = _stage_inputs(inputs)
    res = bass_utils.run_bass_kernel_spmd(
        nc, in_maps, core_ids=list(range(N_CORES)), trace=trace)
    cat = lambda k: np.concatenate(  # noqa: E731
        [res.results[r][k] for r in range(N_CORES)])
    out = (cat("x_out").astype(np.float32), cat("x_attn").astype(np.float32),
           cat("pred_R").astype(np.float32), cat("res_T").astype(np.float32))
    return out, res


def kernel(**inputs):
    out, _ = _run(inputs, trace=False)
    return out
